# revision 22
# baseline (speedup 1.0000x reference)
"""Mixtral decoder layer (attention + top-2-of-8 MoE) on 8 trn2 NeuronCores.

Sharding: attention is head-parallel (8 heads -> 1 head/core, GQA kv head =
core//4), o_proj is sharded over output rows; two 1MB AllGathers knit the
cores back together.  The MoE is expert-parallel (8 experts -> 1 expert/core);
each core computes its expert's contribution weighted by the dense top-2
router weight and the host sums the 8 partial outputs.

Device layout is transposed throughout: activations live as [feature, token]
so every matmul contraction dim sits on the SBUF partition axis.  The host
pre-transposes weights (and folds the rmsnorm gains into the adjacent weight
matrices) and un-transposes the outputs.

All matmuls run in float32r (full-rate).  Walrus requires every operand of an
fp32r matmul to be *produced* as fp32r, so matmul-feeding tiles are declared
float32r and written through f32r APs; non-matmul consumers read them through
.bitcast(float32).
"""

import numpy as np

T, HID, NH, NKV, HD = 2048, 1024, 8, 2, 128
INTER, NE, TOPK = 3584, 8, 2
EPS, THETA = 1e-5, 10000.0
N_CORES = 8
HT = HID // 128    # 8 h-tiles
TT = T // 128      # 16 t-tiles
ITI = INTER // 128  # 28 i-tiles
TSL = 512          # token-slice for the FFN phase
NTS = T // TSL
SCALE = HD ** -0.5

_CACHE = {}


def _build_nc():
    import concourse.bacc as bacc
    import concourse.tile as tile
    import concourse.mybir as mybir
    from contextlib import ExitStack

    dt = mybir.dt
    f32 = dt.float32
    f32r = dt.float32r
    AF = mybir.ActivationFunctionType
    ALU = mybir.AluOpType
    AX = mybir.AxisListType

    nc = bacc.Bacc("TRN2", target_bir_lowering=False, debug=False,
                   num_devices=N_CORES)

    # ---- DRAM I/O ----  (tensors feeding matmuls are float32r: same bits)
    hT_d = nc.dram_tensor("hT", [HID, T], f32r, kind="ExternalInput")
    hslice_d = nc.dram_tensor("hslice", [128, T], f32, kind="ExternalInput")
    qkvT_d = nc.dram_tensor("qkvT", [HID, 3 * HD], f32r, kind="ExternalInput")
    owT_d = nc.dram_tensor("owT", [HID, 128], f32r, kind="ExternalInput")
    gwT_d = nc.dram_tensor("gwT", [HID, NE], f32, kind="ExternalInput")
    w1T_d = nc.dram_tensor("w1T", [ITI, HID, 128], f32r, kind="ExternalInput")
    w3T_d = nc.dram_tensor("w3T", [ITI, HID, 128], f32r, kind="ExternalInput")
    w2T_d = nc.dram_tensor("w2T", [HT, INTER, 128], f32r, kind="ExternalInput")
    cosq_d = nc.dram_tensor("cosq", [128, T], f32, kind="ExternalInput")
    sinq_d = nc.dram_tensor("sinq", [128, T], f32, kind="ExternalInput")
    cosk_d = nc.dram_tensor("cosk", [128, T], f32, kind="ExternalInput")
    sink_d = nc.dram_tensor("sink", [128, T], f32, kind="ExternalInput")
    trimask_d = nc.dram_tensor("trimask", [128, 128], f32, kind="ExternalInput")
    selmask_d = nc.dram_tensor("selmask", [128, NE], f32, kind="ExternalInput")
    ident_d = nc.dram_tensor("ident", [128, 128], f32r, kind="ExternalInput")
    ones_d = nc.dram_tensor("onescol", [128, 1], f32r, kind="ExternalInput")
    moeT_o = nc.dram_tensor("moeT", [HID, T], f32, kind="ExternalOutput")
    residT_o = nc.dram_tensor("residT", [HID, T], f32, kind="ExternalOutput")

    def r(ap):
        return ap.bitcast(f32r)

    def asf(ap):
        return ap.bitcast(f32)

    with tile.TileContext(nc) as tc, ExitStack() as top:
        cpool = top.enter_context(tc.tile_pool(name="consts", bufs=1))
        ident = cpool.tile([128, 128], f32r, tag="ident")
        nc.sync.dma_start(ident[:], ident_d[:])
        ones = cpool.tile([128, 1], f32r, tag="ones")
        nc.sync.dma_start(ones[:], ones_d[:])

        # DRAM bounce buffers for collectives (outputs Shared)
        dpool = top.enter_context(tc.tile_pool(name="dram", bufs=1,
                                               space="DRAM"))
        cout1 = nc.dram_tensor("cc_out1", [HID, T], f32r, addr_space="Shared")
        cout2 = nc.dram_tensor("cc_out2", [HID, T], f32, addr_space="Shared")

        attn_scope = ExitStack()
        apool = attn_scope.enter_context(tc.tile_pool(name="attn_act",
                                                      bufs=1))
        qhat = apool.tile([128, T], f32r, tag="qhat")
        khat = apool.tile([128, T], f32r, tag="khat")
        vT = apool.tile([128, T], f32r, tag="vT")

        def rmsnorm_scale(src_tiles, src_f32r, pool, pool1, pspool, tag):
            """src_tiles: 8 [128, T] tiles covering HID on partitions.
            Returns s [1, T] sbuf tile: rsqrt(mean_h(x^2) + eps)."""
            ps = pspool.tile([1, T], f32, tag=f"{tag}_ps")
            for ht in range(HT):
                src = src_tiles[ht][:]
                if src_f32r:
                    src = asf(src)
                for nt in range(T // 512):
                    sl = slice(nt * 512, nt * 512 + 512)
                    sq = pool.tile([128, 512], f32r, tag=f"{tag}_sq")
                    nc.scalar.square(sq[:], src[:, sl])
                    nc.tensor.matmul(ps[0:1, sl], ones[:], sq[:],
                                     start=(ht == 0), stop=(ht == HT - 1))
            epst = pool1.tile([1, 1], f32, tag=f"{tag}_eps")
            nc.gpsimd.memset(epst[:], EPS)
            srt = pool1.tile([1, T], f32, tag=f"{tag}_srt")
            nc.scalar.activation(srt[:], ps[0:1, :], AF.Sqrt,
                                 bias=epst[:], scale=1.0 / HID)
            s = pool1.tile([1, T], f32, tag=f"{tag}_s")
            nc.vector.reciprocal(s[:], srt[:])
            return s

        # ============ Phase 1: x1T = hiddenT * rsqrt(mean h^2+eps) =========
        with tc.tile_pool(name="p1", bufs=2) as p1, \
             tc.tile_pool(name="p1c", bufs=1) as p1c, \
             tc.tile_pool(name="p1ps", bufs=1, space="PSUM") as p1ps, \
             tc.tile_pool(name="p1x", bufs=1) as p1x, \
             tc.tile_pool(name="p2ps", bufs=1, space="PSUM") as p2ps:
            hts = []
            for ht in range(HT):
                t_ = p1x.tile([128, T], f32r, tag=f"ht{ht}")
                nc.sync.dma_start(t_[:], hT_d[ht * 128:(ht + 1) * 128, :])
                hts.append(t_)
            s1 = rmsnorm_scale(hts, True, p1, p1c, p1ps, "s1")
            S1 = p1x.tile([128, T], f32, tag="S1")
            nc.gpsimd.partition_broadcast(S1[:], s1[:])
            x1 = hts
            for ht in range(HT):
                # in-place normalize; output written as f32r
                nc.vector.tensor_tensor(x1[ht][:], asf(x1[ht][:]), S1[:],
                                        op=ALU.mult)

            # ============ Phase 2a: qkv + rope ============================
            qkvw = []
            for ht in range(HT):
                t_ = p1c.tile([128, 3 * HD], f32r, tag=f"qkvw{ht}")
                nc.sync.dma_start(t_[:], qkvT_d[ht * 128:(ht + 1) * 128, :])
                qkvw.append(t_)

            def load_rope(cd, sd):
                c_ = p1c.tile([128, T], f32, tag="rope_cos", name="rc")
                nc.sync.dma_start(c_[:], cd[:])
                s_ = p1c.tile([128, T], f32, tag="rope_sin", name="rs")
                nc.sync.dma_start(s_[:], sd[:])
                return c_, s_

            def qkv_mm(col, ps):
                for nt in range(T // 512):
                    sl = slice(nt * 512, nt * 512 + 512)
                    for ht in range(HT):
                        nc.tensor.matmul(
                            ps[:, sl],
                            qkvw[ht][:, col * 128:(col + 1) * 128],
                            x1[ht][:, sl],
                            start=(ht == 0), stop=(ht == HT - 1))

            def rope(ps, cos_t, sinsign_t, dst):
                # rope(x) = x*cos128 + swap_halves(x)*[-sin ; +sin]
                raw = p1c.tile([128, T], f32, tag="rope_raw", name="rr")
                nc.any.tensor_copy(raw[:], ps[:])
                sw = p1c.tile([128, T], f32, tag="rope_sw", name="rw")
                nc.sync.dma_start(sw[0:64, :], raw[64:128, :])
                nc.sync.dma_start(sw[64:128, :], raw[0:64, :])
                for nt in range(4):
                    sl = slice(nt * 512, nt * 512 + 512)
                    t1 = p1.tile([128, 512], f32, tag="rope_t1")
                    t2 = p1.tile([128, 512], f32, tag="rope_t2")
                    nc.vector.tensor_mul(t1[:], raw[:, sl], cos_t[:, sl])
                    nc.vector.tensor_mul(t2[:], sw[:, sl], sinsign_t[:, sl])
                    nc.vector.tensor_tensor(dst[:, sl], t1[:], t2[:],
                                            op=ALU.add)

            psq = p2ps.tile([128, T], f32, tag="qkv")
            qkv_mm(0, psq)
            cq_t, sq_t = load_rope(cosq_d, sinq_d)
            rope(psq, cq_t, sq_t, qhat)
            psk = p2ps.tile([128, T], f32, tag="qkv")
            qkv_mm(1, psk)
            ck_t, sk_t = load_rope(cosk_d, sink_d)
            rope(psk, ck_t, sk_t, khat)
            psv = p2ps.tile([128, T], f32, tag="qkv")
            qkv_mm(2, psv)
            nc.any.tensor_copy(vT[:], psv[:])

        # ============ Phase 2b: scores/softmax/PV =========================
        aop = attn_scope.enter_context(tc.tile_pool(name="aop", bufs=1))
        aoT = aop.tile([128, T], f32r, tag="aoT")
        with tc.tile_pool(name="p2b", bufs=2) as p2b, \
             tc.tile_pool(name="p2bx", bufs=1) as p2bx, \
             tc.tile_pool(name="sps", bufs=1, space="PSUM") as sps, \
             tc.tile_pool(name="tps", bufs=2, space="PSUM") as tps:
            trimask = p2bx.tile([128, 128], f32, tag="trimask")
            nc.sync.dma_start(trimask[:], trimask_d[:])
            vN = []
            for kt in range(TT):
                ps = tps.tile([128, 128], f32r, tag="pt_ps")
                nc.tensor.transpose(ps[:], vT[:, kt * 128:(kt + 1) * 128],
                                    ident[:])
                t_ = p2bx.tile([128, 128], f32r, tag=f"vN{kt}")
                nc.vector.tensor_copy(t_[:], ps[:])
                vN.append(t_)

            for qg in range(TT // 4):
                nkt = 4 * qg + 4
                PTs = [p2bx.tile([128, 512], f32r, tag=f"PT{kt}",
                                 name=f"PT{kt}_{qg}")
                       for kt in range(nkt)]
                rrow = p2b.tile([1, 512], f32, tag="rrow")
                for qi in range(4 * qg, 4 * qg + 4):
                    nk = (qi + 1) * 128
                    pss = sps.tile([128, T], f32, tag="scores")
                    for skx in range(0, nk, 512):
                        wk = min(512, nk - skx)
                        nc.tensor.matmul(
                            pss[:, skx:skx + wk],
                            qhat[:, qi * 128:(qi + 1) * 128],
                            khat[:, skx:skx + wk],
                            start=True, stop=True)
                    dsl = slice(qi * 128, (qi + 1) * 128)
                    nc.vector.tensor_add(pss[:, dsl], pss[:, dsl],
                                         trimask[:])
                    nm = p2b.tile([128, 1], f32, tag="nm")
                    nc.vector.reduce_max(nm[:], pss[:, 0:nk], axis=AX.X,
                                         negate=True)
                    probs = p2b.tile([128, T], f32r, tag="probs")
                    rsum = p2b.tile([128, 1], f32, tag="rsum")
                    nc.scalar.activation(probs[:, 0:nk], pss[:, 0:nk],
                                         AF.Exp, bias=nm[:], scale=1.0,
                                         accum_out=rsum[:])
                    rinv = p2b.tile([128, 1], f32, tag="rinv")
                    nc.vector.reciprocal(rinv[:], rsum[:])
                    rinv_r = p2b.tile([128, 1], f32r, tag="rinv_r")
                    nc.vector.tensor_copy(rinv_r[:], rinv[:])
                    pr = tps.tile([1, 128], f32r, tag="pt_ps")
                    nc.tensor.transpose(pr[:], rinv_r[:], ident[:])
                    nc.vector.tensor_copy(
                        asf(rrow[0:1, (qi % 4) * 128:(qi % 4 + 1) * 128]),
                        asf(pr[:]))
                    for kt in range(qi + 1):
                        pt = tps.tile([128, 128], f32r, tag="pt_ps")
                        nc.tensor.transpose(
                            pt[:], probs[:, kt * 128:(kt + 1) * 128],
                            ident[:])
                        nc.vector.tensor_copy(
                            PTs[kt][:, (qi % 4) * 128:(qi % 4 + 1) * 128],
                            pt[:])
                RB = p2b.tile([128, 512], f32, tag="RB")
                nc.gpsimd.partition_broadcast(RB[:], rrow[:])
                pso = tps.tile([128, 512], f32, tag="av")
                for kt in range(nkt):
                    c0 = max(0, (kt - 4 * qg) * 128)
                    nc.tensor.matmul(pso[:, c0:512], vN[kt][:],
                                     PTs[kt][:, c0:512],
                                     start=(kt == 0), stop=(kt == nkt - 1),
                                     skip_group_check=True)
                nc.vector.tensor_tensor(aoT[:, qg * 512:(qg + 1) * 512],
                                        pso[:], RB[:], op=ALU.mult)

        # ============ Phase 2c: AllGather heads + o_proj + residual =======
        with tc.tile_pool(name="p2c", bufs=1) as p2c, \
             tc.tile_pool(name="ops", bufs=1, space="PSUM") as ops:
            cin1 = dpool.tile([128, T], f32r, tag="cin1")
            nc.sync.dma_start(cin1[:], aoT[:])
            nc.gpsimd.collective_compute(
                "AllGather", mybir.AluOpType.bypass,
                replica_groups=[list(range(N_CORES))],
                ins=[cin1.opt()], outs=[cout1.ap()])
            aof = []
            for rt in range(HT):
                t_ = p2c.tile([128, T], f32r, tag=f"aof{rt}")
                nc.sync.dma_start(t_[:], cout1[rt * 128:(rt + 1) * 128, :])
                aof.append(t_)
            oww = []
            for rt in range(HT):
                t_ = p2c.tile([128, 128], f32r, tag=f"oww{rt}")
                nc.sync.dma_start(t_[:], owT_d[rt * 128:(rt + 1) * 128, :])
                oww.append(t_)
            pso = ops.tile([128, T], f32, tag="o")
            for nt in range(T // 512):
                sl = slice(nt * 512, nt * 512 + 512)
                for rt in range(HT):
                    nc.tensor.matmul(pso[:, sl], oww[rt][:],
                                     aof[rt][:, sl],
                                     start=(rt == 0), stop=(rt == HT - 1))
            hsl = p2c.tile([128, T], f32, tag="hsl")
            nc.sync.dma_start(hsl[:], hslice_d[:])
            rsl = p2c.tile([128, T], f32, tag="rsl")
            nc.vector.tensor_add(rsl[:], pso[:], hsl[:])
            cin2 = dpool.tile([128, T], f32, tag="cin2")
            nc.sync.dma_start(cin2[:], rsl[:])
            nc.gpsimd.collective_compute(
                "AllGather", mybir.AluOpType.bypass,
                replica_groups=[list(range(N_CORES))],
                ins=[cin2.opt()], outs=[cout2.ap()])
            nc.sync.dma_start(residT_o[:], cout2.ap())

        attn_scope.close()

        # ============ Phase 3: x2T + fp32 router logits ===================
        x2pool = top.enter_context(tc.tile_pool(name="x2", bufs=1))
        lgpool = top.enter_context(tc.tile_pool(name="lgp", bufs=1))
        lgT = lgpool.tile([NE, T], f32, tag="lgT")
        x2 = []
        with tc.tile_pool(name="p3", bufs=2) as p3, \
             tc.tile_pool(name="p3x", bufs=1) as p3x, \
             tc.tile_pool(name="p3ps", bufs=1, space="PSUM") as p3ps, \
             tc.tile_pool(name="p3lps", bufs=1, space="PSUM") as p3lps:
            rts = []
            for ht in range(HT):
                t_ = p3x.tile([128, T], f32, tag=f"rt{ht}")
                nc.sync.dma_start(t_[:], cout2[ht * 128:(ht + 1) * 128, :])
                rts.append(t_)
            s2 = rmsnorm_scale(rts, False, p3, p3x, p3ps, "s2")
            S2 = p3x.tile([128, T], f32, tag="S2")
            nc.gpsimd.partition_broadcast(S2[:], s2[:])
            # router logits in full fp32 on the *unnormalized* residual
            # (the rmsnorm scale s2[t] commutes with the h-contraction)
            gww = []
            for ht in range(HT):
                t_ = p3x.tile([128, NE], f32, tag=f"gww{ht}")
                nc.sync.dma_start(t_[:], gwT_d[ht * 128:(ht + 1) * 128, :])
                gww.append(t_)
            psl = p3lps.tile([NE, T], f32, tag="lg")
            for nt in range(T // 512):
                sl = slice(nt * 512, nt * 512 + 512)
                for ht in range(HT):
                    nc.tensor.matmul(psl[:, sl], gww[ht][:],
                                     rts[ht][:, sl],
                                     start=(ht == 0), stop=(ht == HT - 1))
            s2b8 = p3x.tile([NE, T], f32, tag="s2b8")
            nc.gpsimd.partition_broadcast(s2b8[:], s2[:])
            nc.vector.tensor_tensor(lgT[:], psl[:], s2b8[:], op=ALU.mult)
            for ht in range(HT):
                t_ = x2pool.tile([128, T], f32r, tag=f"x2{ht}")
                nc.vector.tensor_tensor(t_[:], rts[ht][:], S2[:],
                                        op=ALU.mult)
                x2.append(t_)

        # ============ Phase 4: router -> CW [128, T] ======================
        cwpool = top.enter_context(tc.tile_pool(name="cw", bufs=1))
        CW = cwpool.tile([128, T], f32, tag="CW")
        with tc.tile_pool(name="p4", bufs=2) as p4, \
             tc.tile_pool(name="p4x", bufs=1) as p4x, \
             tc.tile_pool(name="p4tps", bufs=2, space="PSUM") as p4tps:
            selm = p4x.tile([128, NE], f32, tag="selm")
            nc.sync.dma_start(selm[:], selmask_d[:])
            cwrow = p4x.tile([1, T], f32, tag="cwrow")
            for tt in range(TT):
                tsl = slice(tt * 128, (tt + 1) * 128)
                p8 = p4tps.tile([128, NE], f32, tag="lgt_ps")
                nc.tensor.transpose(p8[:], lgT[:, tsl],
                                    asf(ident[0:NE, 0:NE]))
                lgN = p4.tile([128, NE], f32, tag="lgN")
                nc.vector.tensor_copy(lgN[:], p8[:])
                m1 = p4.tile([128, 1], f32, tag="m1")
                nc.vector.reduce_max(m1[:], lgN[:], axis=AX.X)
                eq = p4.tile([128, NE], f32, tag="eq")
                nc.vector.tensor_scalar(eq[:], lgN[:], m1[:], None,
                                        op0=ALU.is_equal)
                msk = p4.tile([128, NE], f32, tag="msk")
                nc.vector.scalar_tensor_tensor(
                    msk[:], eq[:], -1e30, lgN[:],
                    op0=ALU.mult, op1=ALU.add)
                m2 = p4.tile([128, 1], f32, tag="m2")
                nc.vector.reduce_max(m2[:], msk[:], axis=AX.X)
                fl = p4.tile([128, NE], f32, tag="fl")
                nc.vector.tensor_scalar(fl[:], lgN[:], m2[:], None,
                                        op0=ALU.is_ge)
                dd = p4.tile([128, NE], f32, tag="dd")
                nc.vector.tensor_scalar(dd[:], lgN[:], m1[:], None,
                                        op0=ALU.subtract)
                e1 = p4.tile([128, NE], f32, tag="e1")
                nc.scalar.activation(e1[:], dd[:], AF.Exp)
                se = p4.tile([128, NE], f32, tag="se")
                den = p4.tile([128, 1], f32, tag="den")
                nc.vector.scalar_tensor_tensor(se[:], fl[:], 1.0, e1[:],
                                               op0=ALU.mult, op1=ALU.mult,
                                               accum_out=den[:])
                rv = p4.tile([128, 1], f32, tag="rv")
                nc.vector.reciprocal(rv[:], den[:])
                csel = p4.tile([128, NE], f32, tag="csel")
                cws = p4.tile([128, 1], f32, tag="cws")
                nc.vector.scalar_tensor_tensor(csel[:], se[:], 1.0, selm[:],
                                               op0=ALU.mult, op1=ALU.mult,
                                               accum_out=cws[:])
                cwn = p4.tile([128, 1], f32r, tag="cwn")
                nc.vector.tensor_tensor(cwn[:], cws[:], rv[:], op=ALU.mult)
                pc = p4tps.tile([1, 128], f32r, tag="cw_ps")
                nc.tensor.transpose(pc[:], cwn[:], ident[:])
                nc.vector.tensor_copy(asf(cwrow[0:1, tsl]), asf(pc[:]))
            nc.gpsimd.partition_broadcast(CW[:], cwrow[:])

        # ============ Phase 5: expert FFN =================================
        with tc.tile_pool(name="p5w", bufs=2) as p5w, \
             tc.tile_pool(name="p5h", bufs=1) as p5h, \
             tc.tile_pool(name="p5s", bufs=2) as p5s, \
             tc.tile_pool(name="p5ps", bufs=2, space="PSUM") as p5ps:
            for ts in range(NTS):
                sl = slice(ts * TSL, (ts + 1) * TSL)
                hts_ = []
                for it in range(ITI):
                    w1b = p5w.tile([128, HT, 128], f32r, tag="w1b")
                    nc.sync.dma_start(
                        w1b[:],
                        w1T_d[it].rearrange("(a p) f -> p a f", p=128))
                    w3b = p5w.tile([128, HT, 128], f32r, tag="w3b")
                    nc.sync.dma_start(
                        w3b[:],
                        w3T_d[it].rearrange("(a p) f -> p a f", p=128))
                    pg = p5ps.tile([128, TSL], f32, tag="g")
                    pu = p5ps.tile([128, TSL], f32, tag="u")
                    for ht in range(HT):
                        nc.tensor.matmul(pg[:], w1b[:, ht, :],
                                         x2[ht][:, sl],
                                         start=(ht == 0),
                                         stop=(ht == HT - 1))
                    for ht in range(HT):
                        nc.tensor.matmul(pu[:], w3b[:, ht, :],
                                         x2[ht][:, sl],
                                         start=(ht == 0),
                                         stop=(ht == HT - 1))
                    sg = p5s.tile([128, TSL], f32, tag="sg")
                    nc.scalar.activation(sg[:], pg[:], AF.Silu)
                    ht_ = p5h.tile([128, TSL], f32r, tag=f"h{it}")
                    nc.vector.tensor_tensor(ht_[:], sg[:], pu[:],
                                            op=ALU.mult)
                    hts_.append(ht_)
                for dtt in range(HT):
                    w2b = p5w.tile([128, ITI, 128], f32r, tag="w2b")
                    nc.sync.dma_start(
                        w2b[:],
                        w2T_d[dtt].rearrange("(a p) f -> p a f", p=128))
                    pd = p5ps.tile([128, TSL], f32, tag="d")
                    for it in range(ITI):
                        nc.tensor.matmul(pd[:], w2b[:, it, :],
                                         hts_[it][:],
                                         start=(it == 0),
                                         stop=(it == ITI - 1))
                    mo = p5s.tile([128, TSL], f32, tag="mo")
                    nc.vector.tensor_tensor(mo[:], pd[:], CW[:, sl],
                                            op=ALU.mult)
                    nc.sync.dma_start(
                        moeT_o[dtt * 128:(dtt + 1) * 128, sl], mo[:])

    nc.compile()
    return nc


def _host_prep(positions, hidden_states, qkv_w, o_w, gate_w, w1, w2, w3,
               ln1_w, ln2_w):
    f = np.float32
    pos = np.asarray(positions).astype(f)
    hidden = np.asarray(hidden_states, dtype=f)
    hT = np.ascontiguousarray(hidden.T)

    half = HD // 2
    inv = THETA ** (-np.arange(half, dtype=f) * 2.0 / HD)
    ang = inv[:, None] * pos[None, :]          # [64, T]
    cos = np.cos(ang).astype(f)
    sin = np.sin(ang).astype(f)
    cos128 = np.concatenate([cos, cos], axis=0)
    # rope(x) = x*cos128 + swap_halves(x)*sinsign, sinsign = [-sin ; +sin]
    sinsign = np.concatenate([-sin, sin], axis=0)
    cosq, sinq = (cos128 * SCALE).astype(f), (sinsign * SCALE).astype(f)
    cosk, sink = cos128.astype(f), sinsign.astype(f)

    qq, kk = np.meshgrid(np.arange(128), np.arange(128), indexing="ij")
    trimask = np.where(kk <= qq, 0.0, -1e30).astype(f)
    ident = np.eye(128, dtype=f)
    ones = np.ones((128, 1), dtype=f)

    qkv_f = (np.asarray(qkv_w, dtype=f) * np.asarray(ln1_w, dtype=f)[None, :])
    gate_f = (np.asarray(gate_w, dtype=f) * np.asarray(ln2_w, dtype=f)[None, :])
    gwT = np.ascontiguousarray(gate_f.T)       # [HID, 8]
    ln2 = np.asarray(ln2_w, dtype=f)

    in_maps = []
    for c in range(N_CORES):
        kvh = c // (NH // NKV)
        qs = qkv_f[c * HD:(c + 1) * HD]                       # [128, HID]
        ks = qkv_f[NH * HD + kvh * HD: NH * HD + (kvh + 1) * HD]
        vs = qkv_f[(NH + NKV) * HD + kvh * HD:
                   (NH + NKV) * HD + (kvh + 1) * HD]
        qkvT = np.ascontiguousarray(
            np.concatenate([qs, ks, vs], axis=0).T)           # [HID, 384]
        owT = np.ascontiguousarray(
            np.asarray(o_w, dtype=f)[c * 128:(c + 1) * 128, :].T)  # [HID,128]
        w1c = np.asarray(w1[c], dtype=f) * ln2[None, :]       # [INTER, HID]
        w3c = np.asarray(w3[c], dtype=f) * ln2[None, :]
        w2c = np.asarray(w2[c], dtype=f)                      # [HID, INTER]
        w1T = np.ascontiguousarray(
            w1c.T.reshape(HID, ITI, 128).transpose(1, 0, 2))  # [28, HID, 128]
        w3T = np.ascontiguousarray(
            w3c.T.reshape(HID, ITI, 128).transpose(1, 0, 2))
        w2T = np.ascontiguousarray(
            w2c.T.reshape(INTER, HT, 128).transpose(1, 0, 2))  # [8,INTER,128]
        selmask = np.zeros((128, NE), dtype=f)
        selmask[:, c] = 1.0
        in_maps.append({
            "hT": hT,
            "hslice": np.ascontiguousarray(hT[c * 128:(c + 1) * 128]),
            "qkvT": qkvT, "owT": owT, "gwT": gwT,
            "w1T": w1T, "w3T": w3T, "w2T": w2T,
            "cosq": cosq, "sinq": sinq, "cosk": cosk, "sink": sink,
            "trimask": trimask, "selmask": selmask, "ident": ident,
            "onescol": ones,
        })
    return in_maps


def kernel(positions, hidden_states, qkv_w, o_w, gate_w, w1, w2, w3,
           ln1_w, ln2_w, _trace=False):
    from concourse.bass_utils import run_bass_kernel_spmd
    if "nc" not in _CACHE:
        _CACHE["nc"] = _build_nc()
    nc = _CACHE["nc"]
    in_maps = _host_prep(positions, hidden_states, qkv_w, o_w, gate_w,
                         w1, w2, w3, ln1_w, ln2_w)
    res = run_bass_kernel_spmd(nc, in_maps, list(range(N_CORES)),
                               trace=_trace)
    _CACHE["last_result"] = res
    moeT = np.zeros((HID, T), dtype=np.float64)
    for c in range(N_CORES):
        moeT += res.results[c]["moeT"]
    moe = np.ascontiguousarray(moeT.T).astype(np.float32)
    resid = np.ascontiguousarray(res.results[0]["residT"].T)
    return (moe, resid)


# revision 23
# speedup vs baseline: 1.0598x; 1.0598x over previous
"""Mixtral decoder layer (attention + top-2-of-8 MoE) on 8 trn2 NeuronCores.

Sharding: attention is head-parallel (8 heads -> 1 head/core, GQA kv head =
core//4), o_proj is sharded over output rows; two 1MB AllGathers knit the
cores back together.  The MoE is expert-parallel (8 experts -> 1 expert/core);
each core computes its expert's contribution weighted by the dense top-2
router weight and the host sums the 8 partial outputs.

Device layout is transposed throughout: activations live as [feature, token]
so every matmul contraction dim sits on the SBUF partition axis.  The host
pre-transposes weights (and folds the rmsnorm gains into the adjacent weight
matrices) and un-transposes the outputs.

All matmuls run in float32r (full-rate).  Walrus requires every operand of an
fp32r matmul to be *produced* as fp32r, so matmul-feeding tiles are declared
float32r and written through f32r APs; non-matmul consumers read them through
.bitcast(float32).
"""

import numpy as np

T, HID, NH, NKV, HD = 2048, 1024, 8, 2, 128
INTER, NE, TOPK = 3584, 8, 2
EPS, THETA = 1e-5, 10000.0
N_CORES = 8
HT = HID // 128    # 8 h-tiles
TT = T // 128      # 16 t-tiles
ITI = INTER // 128  # 28 i-tiles
TSL = 512          # token-slice for the FFN phase
NTS = T // TSL
SCALE = HD ** -0.5

_CACHE = {}


def _build_nc():
    import concourse.bacc as bacc
    import concourse.tile as tile
    import concourse.mybir as mybir
    from contextlib import ExitStack

    dt = mybir.dt
    f32 = dt.float32
    f32r = dt.float32r
    AF = mybir.ActivationFunctionType
    ALU = mybir.AluOpType
    AX = mybir.AxisListType

    nc = bacc.Bacc("TRN2", target_bir_lowering=False, debug=False,
                   num_devices=N_CORES)

    # ---- DRAM I/O ----  (tensors feeding matmuls are float32r: same bits)
    hT_d = nc.dram_tensor("hT", [HID, T], f32r, kind="ExternalInput")
    hslice_d = nc.dram_tensor("hslice", [128, T], f32, kind="ExternalInput")
    qkvT_d = nc.dram_tensor("qkvT", [HID, 3 * HD], f32r, kind="ExternalInput")
    owT_d = nc.dram_tensor("owT", [HID, 128], f32r, kind="ExternalInput")
    gwT_d = nc.dram_tensor("gwT", [HID, NE], f32, kind="ExternalInput")
    w1T_d = nc.dram_tensor("w1T", [ITI, 128, HT, 128], f32r,
                           kind="ExternalInput")
    w3T_d = nc.dram_tensor("w3T", [ITI, 128, HT, 128], f32r,
                           kind="ExternalInput")
    w2T_d = nc.dram_tensor("w2T", [HT, 128, ITI, 128], f32r,
                           kind="ExternalInput")
    cosq_d = nc.dram_tensor("cosq", [128, T], f32, kind="ExternalInput")
    sinq_d = nc.dram_tensor("sinq", [128, T], f32, kind="ExternalInput")
    cosk_d = nc.dram_tensor("cosk", [128, T], f32, kind="ExternalInput")
    sink_d = nc.dram_tensor("sink", [128, T], f32, kind="ExternalInput")
    trimask_d = nc.dram_tensor("trimask", [128, 128], f32, kind="ExternalInput")
    selmask_d = nc.dram_tensor("selmask", [128, NE], f32, kind="ExternalInput")
    ident_d = nc.dram_tensor("ident", [128, 128], f32r, kind="ExternalInput")
    ones_d = nc.dram_tensor("onescol", [128, 1], f32r, kind="ExternalInput")
    moeT_o = nc.dram_tensor("moeT", [HID, T], f32, kind="ExternalOutput")
    residT_o = nc.dram_tensor("residT", [HID, T], f32, kind="ExternalOutput")

    def r(ap):
        return ap.bitcast(f32r)

    def asf(ap):
        return ap.bitcast(f32)

    with tile.TileContext(nc) as tc, ExitStack() as top:
        cpool = top.enter_context(tc.tile_pool(name="consts", bufs=1))
        ident = cpool.tile([128, 128], f32r, tag="ident")
        nc.sync.dma_start(ident[:], ident_d[:])
        ones = cpool.tile([128, 1], f32r, tag="ones")
        nc.sync.dma_start(ones[:], ones_d[:])

        # DRAM bounce buffers for collectives (outputs Shared)
        dpool = top.enter_context(tc.tile_pool(name="dram", bufs=1,
                                               space="DRAM"))
        cout1 = nc.dram_tensor("cc_out1", [HID, T], f32r, addr_space="Shared")
        cout2 = nc.dram_tensor("cc_out2", [HID, T], f32, addr_space="Shared")

        attn_scope = ExitStack()
        apool = attn_scope.enter_context(tc.tile_pool(name="attn_act",
                                                      bufs=1))
        qhat = apool.tile([128, T], f32r, tag="qhat")
        khat = apool.tile([128, T], f32r, tag="khat")
        vT = apool.tile([128, T], f32r, tag="vT")

        def rmsnorm_scale(src_tiles, src_f32r, pool, pool1, pspool, tag):
            """src_tiles: 8 [128, T] tiles covering HID on partitions.
            Returns s [1, T] sbuf tile: rsqrt(mean_h(x^2) + eps)."""
            ps = pspool.tile([1, T], f32, tag=f"{tag}_ps")
            for ht in range(HT):
                src = src_tiles[ht][:]
                if src_f32r:
                    src = asf(src)
                for nt in range(T // 512):
                    sl = slice(nt * 512, nt * 512 + 512)
                    sq = pool.tile([128, 512], f32r, tag=f"{tag}_sq")
                    nc.scalar.square(sq[:], src[:, sl])
                    nc.tensor.matmul(ps[0:1, sl], ones[:], sq[:],
                                     start=(ht == 0), stop=(ht == HT - 1))
            epst = pool1.tile([1, 1], f32, tag=f"{tag}_eps")
            nc.gpsimd.memset(epst[:], EPS)
            srt = pool1.tile([1, T], f32, tag=f"{tag}_srt")
            nc.scalar.activation(srt[:], ps[0:1, :], AF.Sqrt,
                                 bias=epst[:], scale=1.0 / HID)
            s = pool1.tile([1, T], f32, tag=f"{tag}_s")
            nc.vector.reciprocal(s[:], srt[:])
            return s

        # ============ Phase 1: x1T = hiddenT * rsqrt(mean h^2+eps) =========
        with tc.tile_pool(name="p1", bufs=2) as p1, \
             tc.tile_pool(name="p1c", bufs=1) as p1c, \
             tc.tile_pool(name="p1ps", bufs=1, space="PSUM") as p1ps, \
             tc.tile_pool(name="p1x", bufs=1) as p1x, \
             tc.tile_pool(name="p2ps", bufs=1, space="PSUM") as p2ps:
            hts = []
            for ht in range(HT):
                t_ = p1x.tile([128, T], f32r, tag=f"ht{ht}")
                nc.sync.dma_start(t_[:], hT_d[ht * 128:(ht + 1) * 128, :])
                hts.append(t_)
            s1 = rmsnorm_scale(hts, True, p1, p1c, p1ps, "s1")
            S1 = p1x.tile([128, T], f32, tag="S1")
            nc.gpsimd.partition_broadcast(S1[:], s1[:])
            x1 = hts
            for ht in range(HT):
                # in-place normalize; output written as f32r
                nc.vector.tensor_tensor(x1[ht][:], asf(x1[ht][:]), S1[:],
                                        op=ALU.mult)

            # ============ Phase 2a: qkv + rope ============================
            qkvw = []
            for ht in range(HT):
                t_ = p1c.tile([128, 3 * HD], f32r, tag=f"qkvw{ht}")
                nc.sync.dma_start(t_[:], qkvT_d[ht * 128:(ht + 1) * 128, :])
                qkvw.append(t_)

            def load_rope(cd, sd):
                c_ = p1c.tile([128, T], f32, tag="rope_cos", name="rc")
                nc.sync.dma_start(c_[:], cd[:])
                s_ = p1c.tile([128, T], f32, tag="rope_sin", name="rs")
                nc.sync.dma_start(s_[:], sd[:])
                return c_, s_

            def qkv_mm(col, ps):
                for nt in range(T // 512):
                    sl = slice(nt * 512, nt * 512 + 512)
                    for ht in range(HT):
                        nc.tensor.matmul(
                            ps[:, sl],
                            qkvw[ht][:, col * 128:(col + 1) * 128],
                            x1[ht][:, sl],
                            start=(ht == 0), stop=(ht == HT - 1))

            def rope(ps, cos_t, sinsign_t, dst):
                # rope(x) = x*cos128 + swap_halves(x)*[-sin ; +sin]
                raw = p1c.tile([128, T], f32, tag="rope_raw", name="rr")
                nc.any.tensor_copy(raw[:], ps[:])
                sw = p1c.tile([128, T], f32, tag="rope_sw", name="rw")
                nc.sync.dma_start(sw[0:64, :], raw[64:128, :])
                nc.sync.dma_start(sw[64:128, :], raw[0:64, :])
                for nt in range(4):
                    sl = slice(nt * 512, nt * 512 + 512)
                    t1 = p1.tile([128, 512], f32, tag="rope_t1")
                    t2 = p1.tile([128, 512], f32, tag="rope_t2")
                    nc.vector.tensor_mul(t1[:], raw[:, sl], cos_t[:, sl])
                    nc.vector.tensor_mul(t2[:], sw[:, sl], sinsign_t[:, sl])
                    nc.vector.tensor_tensor(dst[:, sl], t1[:], t2[:],
                                            op=ALU.add)

            psq = p2ps.tile([128, T], f32, tag="qkv")
            qkv_mm(0, psq)
            cq_t, sq_t = load_rope(cosq_d, sinq_d)
            rope(psq, cq_t, sq_t, qhat)
            psk = p2ps.tile([128, T], f32, tag="qkv")
            qkv_mm(1, psk)
            ck_t, sk_t = load_rope(cosk_d, sink_d)
            rope(psk, ck_t, sk_t, khat)
            psv = p2ps.tile([128, T], f32, tag="qkv")
            qkv_mm(2, psv)
            nc.any.tensor_copy(vT[:], psv[:])

        # ============ Phase 2b: scores/softmax/PV =========================
        aop = attn_scope.enter_context(tc.tile_pool(name="aop", bufs=1))
        aoT = aop.tile([128, T], f32r, tag="aoT")
        with tc.tile_pool(name="p2b", bufs=2) as p2b, \
             tc.tile_pool(name="p2bx", bufs=1) as p2bx, \
             tc.tile_pool(name="sps", bufs=1, space="PSUM") as sps, \
             tc.tile_pool(name="tps", bufs=2, space="PSUM") as tps:
            trimask = p2bx.tile([128, 128], f32, tag="trimask")
            nc.sync.dma_start(trimask[:], trimask_d[:])
            vN = []
            for kt in range(TT):
                ps = tps.tile([128, 128], f32r, tag="pt_ps")
                nc.tensor.transpose(ps[:], vT[:, kt * 128:(kt + 1) * 128],
                                    ident[:])
                t_ = p2bx.tile([128, 128], f32r, tag=f"vN{kt}")
                nc.vector.tensor_copy(t_[:], ps[:])
                vN.append(t_)

            for qg in range(TT // 4):
                nkt = 4 * qg + 4
                PTs = [p2bx.tile([128, 512], f32r, tag=f"PT{kt}",
                                 name=f"PT{kt}_{qg}")
                       for kt in range(nkt)]
                rrow = p2b.tile([1, 512], f32, tag="rrow")
                for qi in range(4 * qg, 4 * qg + 4):
                    nk = (qi + 1) * 128
                    pss = sps.tile([128, T], f32, tag="scores")
                    for skx in range(0, nk, 512):
                        wk = min(512, nk - skx)
                        nc.tensor.matmul(
                            pss[:, skx:skx + wk],
                            qhat[:, qi * 128:(qi + 1) * 128],
                            khat[:, skx:skx + wk],
                            start=True, stop=True)
                    dsl = slice(qi * 128, (qi + 1) * 128)
                    nc.vector.tensor_add(pss[:, dsl], pss[:, dsl],
                                         trimask[:])
                    nm = p2b.tile([128, 1], f32, tag="nm")
                    nc.vector.reduce_max(nm[:], pss[:, 0:nk], axis=AX.X,
                                         negate=True)
                    probs = p2b.tile([128, T], f32r, tag="probs")
                    rsum = p2b.tile([128, 1], f32, tag="rsum")
                    nc.scalar.activation(probs[:, 0:nk], pss[:, 0:nk],
                                         AF.Exp, bias=nm[:], scale=1.0,
                                         accum_out=rsum[:])
                    rinv = p2b.tile([128, 1], f32, tag="rinv")
                    nc.vector.reciprocal(rinv[:], rsum[:])
                    rinv_r = p2b.tile([128, 1], f32r, tag="rinv_r")
                    nc.vector.tensor_copy(rinv_r[:], rinv[:])
                    pr = tps.tile([1, 128], f32r, tag="pt_ps")
                    nc.tensor.transpose(pr[:], rinv_r[:], ident[:])
                    nc.vector.tensor_copy(
                        asf(rrow[0:1, (qi % 4) * 128:(qi % 4 + 1) * 128]),
                        asf(pr[:]))
                    for kt in range(qi + 1):
                        pt = tps.tile([128, 128], f32r, tag="pt_ps")
                        nc.tensor.transpose(
                            pt[:], probs[:, kt * 128:(kt + 1) * 128],
                            ident[:])
                        nc.vector.tensor_copy(
                            PTs[kt][:, (qi % 4) * 128:(qi % 4 + 1) * 128],
                            pt[:])
                RB = p2b.tile([128, 512], f32, tag="RB")
                nc.gpsimd.partition_broadcast(RB[:], rrow[:])
                pso = tps.tile([128, 512], f32, tag="av")
                for kt in range(nkt):
                    c0 = max(0, (kt - 4 * qg) * 128)
                    nc.tensor.matmul(pso[:, c0:512], vN[kt][:],
                                     PTs[kt][:, c0:512],
                                     start=(kt == 0), stop=(kt == nkt - 1),
                                     skip_group_check=True)
                nc.vector.tensor_tensor(aoT[:, qg * 512:(qg + 1) * 512],
                                        pso[:], RB[:], op=ALU.mult)

        # ============ Phase 2c: AllGather heads + o_proj + residual =======
        with tc.tile_pool(name="p2c", bufs=1) as p2c, \
             tc.tile_pool(name="ops", bufs=1, space="PSUM") as ops:
            cin1 = dpool.tile([128, T], f32r, tag="cin1")
            nc.sync.dma_start(cin1[:], aoT[:])
            nc.gpsimd.collective_compute(
                "AllGather", mybir.AluOpType.bypass,
                replica_groups=[list(range(N_CORES))],
                ins=[cin1.opt()], outs=[cout1.ap()])
            aof = []
            for rt in range(HT):
                t_ = p2c.tile([128, T], f32r, tag=f"aof{rt}")
                nc.sync.dma_start(t_[:], cout1[rt * 128:(rt + 1) * 128, :])
                aof.append(t_)
            oww = []
            for rt in range(HT):
                t_ = p2c.tile([128, 128], f32r, tag=f"oww{rt}")
                nc.sync.dma_start(t_[:], owT_d[rt * 128:(rt + 1) * 128, :])
                oww.append(t_)
            pso = ops.tile([128, T], f32, tag="o")
            for nt in range(T // 512):
                sl = slice(nt * 512, nt * 512 + 512)
                for rt in range(HT):
                    nc.tensor.matmul(pso[:, sl], oww[rt][:],
                                     aof[rt][:, sl],
                                     start=(rt == 0), stop=(rt == HT - 1))
            hsl = p2c.tile([128, T], f32, tag="hsl")
            nc.sync.dma_start(hsl[:], hslice_d[:])
            rsl = p2c.tile([128, T], f32, tag="rsl")
            nc.vector.tensor_add(rsl[:], pso[:], hsl[:])
            cin2 = dpool.tile([128, T], f32, tag="cin2")
            nc.sync.dma_start(cin2[:], rsl[:])
            nc.gpsimd.collective_compute(
                "AllGather", mybir.AluOpType.bypass,
                replica_groups=[list(range(N_CORES))],
                ins=[cin2.opt()], outs=[cout2.ap()])
            nc.sync.dma_start(residT_o[:], cout2.ap())

        attn_scope.close()

        # ============ Phase 3: x2T + fp32 router logits ===================
        x2pool = top.enter_context(tc.tile_pool(name="x2", bufs=1))
        lgpool = top.enter_context(tc.tile_pool(name="lgp", bufs=1))
        lgT = lgpool.tile([NE, T], f32, tag="lgT")
        x2 = []
        with tc.tile_pool(name="p3", bufs=2) as p3, \
             tc.tile_pool(name="p3x", bufs=1) as p3x, \
             tc.tile_pool(name="p3ps", bufs=1, space="PSUM") as p3ps, \
             tc.tile_pool(name="p3lps", bufs=1, space="PSUM") as p3lps:
            rts = []
            for ht in range(HT):
                t_ = p3x.tile([128, T], f32, tag=f"rt{ht}")
                nc.sync.dma_start(t_[:], cout2[ht * 128:(ht + 1) * 128, :])
                rts.append(t_)
            s2 = rmsnorm_scale(rts, False, p3, p3x, p3ps, "s2")
            S2 = p3x.tile([128, T], f32, tag="S2")
            nc.gpsimd.partition_broadcast(S2[:], s2[:])
            # router logits in full fp32 on the *unnormalized* residual
            # (the rmsnorm scale s2[t] commutes with the h-contraction)
            gww = []
            for ht in range(HT):
                t_ = p3x.tile([128, NE], f32, tag=f"gww{ht}")
                nc.sync.dma_start(t_[:], gwT_d[ht * 128:(ht + 1) * 128, :])
                gww.append(t_)
            psl = p3lps.tile([NE, T], f32, tag="lg")
            for nt in range(T // 512):
                sl = slice(nt * 512, nt * 512 + 512)
                for ht in range(HT):
                    nc.tensor.matmul(psl[:, sl], gww[ht][:],
                                     rts[ht][:, sl],
                                     start=(ht == 0), stop=(ht == HT - 1))
            s2b8 = p3x.tile([NE, T], f32, tag="s2b8")
            nc.gpsimd.partition_broadcast(s2b8[:], s2[:])
            nc.vector.tensor_tensor(lgT[:], psl[:], s2b8[:], op=ALU.mult)
            for ht in range(HT):
                t_ = x2pool.tile([128, T], f32r, tag=f"x2{ht}")
                nc.vector.tensor_tensor(t_[:], rts[ht][:], S2[:],
                                        op=ALU.mult)
                x2.append(t_)

        # ============ Phase 4: router -> CW [128, T] ======================
        cwpool = top.enter_context(tc.tile_pool(name="cw", bufs=1))
        CW = cwpool.tile([128, T], f32, tag="CW")
        with tc.tile_pool(name="p4", bufs=2) as p4, \
             tc.tile_pool(name="p4x", bufs=1) as p4x, \
             tc.tile_pool(name="p4tps", bufs=2, space="PSUM") as p4tps:
            selm = p4x.tile([128, NE], f32, tag="selm")
            nc.sync.dma_start(selm[:], selmask_d[:])
            cwrow = p4x.tile([1, T], f32, tag="cwrow")
            for tt in range(TT):
                tsl = slice(tt * 128, (tt + 1) * 128)
                p8 = p4tps.tile([128, NE], f32, tag="lgt_ps")
                nc.tensor.transpose(p8[:], lgT[:, tsl],
                                    asf(ident[0:NE, 0:NE]))
                lgN = p4.tile([128, NE], f32, tag="lgN")
                nc.vector.tensor_copy(lgN[:], p8[:])
                m1 = p4.tile([128, 1], f32, tag="m1")
                nc.vector.reduce_max(m1[:], lgN[:], axis=AX.X)
                eq = p4.tile([128, NE], f32, tag="eq")
                nc.vector.tensor_scalar(eq[:], lgN[:], m1[:], None,
                                        op0=ALU.is_equal)
                msk = p4.tile([128, NE], f32, tag="msk")
                nc.vector.scalar_tensor_tensor(
                    msk[:], eq[:], -1e30, lgN[:],
                    op0=ALU.mult, op1=ALU.add)
                m2 = p4.tile([128, 1], f32, tag="m2")
                nc.vector.reduce_max(m2[:], msk[:], axis=AX.X)
                fl = p4.tile([128, NE], f32, tag="fl")
                nc.vector.tensor_scalar(fl[:], lgN[:], m2[:], None,
                                        op0=ALU.is_ge)
                dd = p4.tile([128, NE], f32, tag="dd")
                nc.vector.tensor_scalar(dd[:], lgN[:], m1[:], None,
                                        op0=ALU.subtract)
                e1 = p4.tile([128, NE], f32, tag="e1")
                nc.scalar.activation(e1[:], dd[:], AF.Exp)
                se = p4.tile([128, NE], f32, tag="se")
                den = p4.tile([128, 1], f32, tag="den")
                nc.vector.scalar_tensor_tensor(se[:], fl[:], 1.0, e1[:],
                                               op0=ALU.mult, op1=ALU.mult,
                                               accum_out=den[:])
                rv = p4.tile([128, 1], f32, tag="rv")
                nc.vector.reciprocal(rv[:], den[:])
                csel = p4.tile([128, NE], f32, tag="csel")
                cws = p4.tile([128, 1], f32, tag="cws")
                nc.vector.scalar_tensor_tensor(csel[:], se[:], 1.0, selm[:],
                                               op0=ALU.mult, op1=ALU.mult,
                                               accum_out=cws[:])
                cwn = p4.tile([128, 1], f32r, tag="cwn")
                nc.vector.tensor_tensor(cwn[:], cws[:], rv[:], op=ALU.mult)
                pc = p4tps.tile([1, 128], f32r, tag="cw_ps")
                nc.tensor.transpose(pc[:], cwn[:], ident[:])
                nc.vector.tensor_copy(asf(cwrow[0:1, tsl]), asf(pc[:]))
            nc.gpsimd.partition_broadcast(CW[:], cwrow[:])

        # ============ Phase 5: expert FFN =================================
        with tc.tile_pool(name="p5w", bufs=2) as p5w, \
             tc.tile_pool(name="p5h", bufs=1) as p5h, \
             tc.tile_pool(name="p5s", bufs=2) as p5s, \
             tc.tile_pool(name="p5ps", bufs=2, space="PSUM") as p5ps:
            for ts in range(NTS):
                sl = slice(ts * TSL, (ts + 1) * TSL)
                hts_ = []
                for it in range(ITI):
                    w1b = p5w.tile([128, HT, 128], f32r, tag="w1b")
                    nc.sync.dma_start(w1b[:], w1T_d[it])
                    w3b = p5w.tile([128, HT, 128], f32r, tag="w3b")
                    nc.sync.dma_start(w3b[:], w3T_d[it])
                    pg = p5ps.tile([128, TSL], f32, tag="g")
                    pu = p5ps.tile([128, TSL], f32, tag="u")
                    for ht in range(HT):
                        nc.tensor.matmul(pg[:], w1b[:, ht, :],
                                         x2[ht][:, sl],
                                         start=(ht == 0),
                                         stop=(ht == HT - 1))
                    for ht in range(HT):
                        nc.tensor.matmul(pu[:], w3b[:, ht, :],
                                         x2[ht][:, sl],
                                         start=(ht == 0),
                                         stop=(ht == HT - 1))
                    sg = p5s.tile([128, TSL], f32, tag="sg")
                    nc.scalar.activation(sg[:], pg[:], AF.Silu)
                    ht_ = p5h.tile([128, TSL], f32r, tag=f"h{it}")
                    nc.vector.tensor_tensor(ht_[:], sg[:], pu[:],
                                            op=ALU.mult)
                    hts_.append(ht_)
                for dtt in range(HT):
                    w2b = p5w.tile([128, ITI, 128], f32r, tag="w2b")
                    nc.sync.dma_start(w2b[:], w2T_d[dtt])
                    pd = p5ps.tile([128, TSL], f32, tag="d")
                    for it in range(ITI):
                        nc.tensor.matmul(pd[:], w2b[:, it, :],
                                         hts_[it][:],
                                         start=(it == 0),
                                         stop=(it == ITI - 1))
                    mo = p5s.tile([128, TSL], f32, tag="mo")
                    nc.vector.tensor_tensor(mo[:], pd[:], CW[:, sl],
                                            op=ALU.mult)
                    nc.sync.dma_start(
                        moeT_o[dtt * 128:(dtt + 1) * 128, sl], mo[:])

    nc.compile()
    return nc


def _host_prep(positions, hidden_states, qkv_w, o_w, gate_w, w1, w2, w3,
               ln1_w, ln2_w):
    f = np.float32
    pos = np.asarray(positions).astype(f)
    hidden = np.asarray(hidden_states, dtype=f)
    hT = np.ascontiguousarray(hidden.T)

    half = HD // 2
    inv = THETA ** (-np.arange(half, dtype=f) * 2.0 / HD)
    ang = inv[:, None] * pos[None, :]          # [64, T]
    cos = np.cos(ang).astype(f)
    sin = np.sin(ang).astype(f)
    cos128 = np.concatenate([cos, cos], axis=0)
    # rope(x) = x*cos128 + swap_halves(x)*sinsign, sinsign = [-sin ; +sin]
    sinsign = np.concatenate([-sin, sin], axis=0)
    cosq, sinq = (cos128 * SCALE).astype(f), (sinsign * SCALE).astype(f)
    cosk, sink = cos128.astype(f), sinsign.astype(f)

    qq, kk = np.meshgrid(np.arange(128), np.arange(128), indexing="ij")
    trimask = np.where(kk <= qq, 0.0, -1e30).astype(f)
    ident = np.eye(128, dtype=f)
    ones = np.ones((128, 1), dtype=f)

    qkv_f = (np.asarray(qkv_w, dtype=f) * np.asarray(ln1_w, dtype=f)[None, :])
    gate_f = (np.asarray(gate_w, dtype=f) * np.asarray(ln2_w, dtype=f)[None, :])
    gwT = np.ascontiguousarray(gate_f.T)       # [HID, 8]
    ln2 = np.asarray(ln2_w, dtype=f)

    in_maps = []
    for c in range(N_CORES):
        kvh = c // (NH // NKV)
        qs = qkv_f[c * HD:(c + 1) * HD]                       # [128, HID]
        ks = qkv_f[NH * HD + kvh * HD: NH * HD + (kvh + 1) * HD]
        vs = qkv_f[(NH + NKV) * HD + kvh * HD:
                   (NH + NKV) * HD + (kvh + 1) * HD]
        qkvT = np.ascontiguousarray(
            np.concatenate([qs, ks, vs], axis=0).T)           # [HID, 384]
        owT = np.ascontiguousarray(
            np.asarray(o_w, dtype=f)[c * 128:(c + 1) * 128, :].T)  # [HID,128]
        w1c = np.asarray(w1[c], dtype=f) * ln2[None, :]       # [INTER, HID]
        w3c = np.asarray(w3[c], dtype=f) * ln2[None, :]
        w2c = np.asarray(w2[c], dtype=f)                      # [HID, INTER]
        # [ITI, 128p, HT, 128f]: lhsT tile [p, a, f] loads contiguously
        w1T = np.ascontiguousarray(
            w1c.T.reshape(HT, 128, ITI, 128).transpose(2, 1, 0, 3))
        w3T = np.ascontiguousarray(
            w3c.T.reshape(HT, 128, ITI, 128).transpose(2, 1, 0, 3))
        w2T = np.ascontiguousarray(
            w2c.T.reshape(ITI, 128, HT, 128).transpose(2, 1, 0, 3))
        selmask = np.zeros((128, NE), dtype=f)
        selmask[:, c] = 1.0
        in_maps.append({
            "hT": hT,
            "hslice": np.ascontiguousarray(hT[c * 128:(c + 1) * 128]),
            "qkvT": qkvT, "owT": owT, "gwT": gwT,
            "w1T": w1T, "w3T": w3T, "w2T": w2T,
            "cosq": cosq, "sinq": sinq, "cosk": cosk, "sink": sink,
            "trimask": trimask, "selmask": selmask, "ident": ident,
            "onescol": ones,
        })
    return in_maps


def kernel(positions, hidden_states, qkv_w, o_w, gate_w, w1, w2, w3,
           ln1_w, ln2_w, _trace=False):
    from concourse.bass_utils import run_bass_kernel_spmd
    if "nc" not in _CACHE:
        _CACHE["nc"] = _build_nc()
    nc = _CACHE["nc"]
    in_maps = _host_prep(positions, hidden_states, qkv_w, o_w, gate_w,
                         w1, w2, w3, ln1_w, ln2_w)
    res = run_bass_kernel_spmd(nc, in_maps, list(range(N_CORES)),
                               trace=_trace)
    _CACHE["last_result"] = res
    moeT = np.zeros((HID, T), dtype=np.float64)
    for c in range(N_CORES):
        moeT += res.results[c]["moeT"]
    moe = np.ascontiguousarray(moeT.T).astype(np.float32)
    resid = np.ascontiguousarray(res.results[0]["residT"].T)
    return (moe, resid)


# revision 25
# speedup vs baseline: 1.0658x; 1.0057x over previous
"""Mixtral decoder layer (attention + top-2-of-8 MoE) on 8 trn2 NeuronCores.

Sharding: attention is head-parallel (8 heads -> 1 head/core, GQA kv head =
core//4), o_proj is sharded over output rows; two 1MB AllGathers knit the
cores back together.  The MoE is expert-parallel (8 experts -> 1 expert/core);
each core computes its expert's contribution weighted by the dense top-2
router weight and the host sums the 8 partial outputs.

Device layout is transposed throughout: activations live as [feature, token]
so every matmul contraction dim sits on the SBUF partition axis.  The host
pre-transposes weights (and folds the rmsnorm gains into the adjacent weight
matrices) and un-transposes the outputs.

All matmuls run in float32r (full-rate).  Walrus requires every operand of an
fp32r matmul to be *produced* as fp32r, so matmul-feeding tiles are declared
float32r and written through f32r APs; non-matmul consumers read them through
.bitcast(float32).
"""

import numpy as np

T, HID, NH, NKV, HD = 2048, 1024, 8, 2, 128
INTER, NE, TOPK = 3584, 8, 2
EPS, THETA = 1e-5, 10000.0
N_CORES = 8
HT = HID // 128    # 8 h-tiles
TT = T // 128      # 16 t-tiles
ITI = INTER // 128  # 28 i-tiles
TSL = 512          # token-slice for the FFN phase
NTS = T // TSL
SCALE = HD ** -0.5

_CACHE = {}


def _build_nc():
    import concourse.bacc as bacc
    import concourse.tile as tile
    import concourse.mybir as mybir
    from contextlib import ExitStack

    dt = mybir.dt
    f32 = dt.float32
    f32r = dt.float32r
    AF = mybir.ActivationFunctionType
    ALU = mybir.AluOpType
    AX = mybir.AxisListType

    nc = bacc.Bacc("TRN2", target_bir_lowering=False, debug=False,
                   num_devices=N_CORES)

    # ---- DRAM I/O ----  (tensors feeding matmuls are float32r: same bits)
    hT_d = nc.dram_tensor("hT", [HID, T], f32r, kind="ExternalInput")
    hslice_d = nc.dram_tensor("hslice", [128, T], f32, kind="ExternalInput")
    qkvT_d = nc.dram_tensor("qkvT", [HID, 3 * HD], f32r, kind="ExternalInput")
    owT_d = nc.dram_tensor("owT", [HID, 128], f32r, kind="ExternalInput")
    gwT_d = nc.dram_tensor("gwT", [HID, NE], f32, kind="ExternalInput")
    w1T_d = nc.dram_tensor("w1T", [ITI, 128, HT, 128], f32r,
                           kind="ExternalInput")
    w3T_d = nc.dram_tensor("w3T", [ITI, 128, HT, 128], f32r,
                           kind="ExternalInput")
    w2T_d = nc.dram_tensor("w2T", [HT, 128, ITI, 128], f32r,
                           kind="ExternalInput")
    cosq_d = nc.dram_tensor("cosq", [128, T], f32, kind="ExternalInput")
    sinq_d = nc.dram_tensor("sinq", [128, T], f32, kind="ExternalInput")
    cosk_d = nc.dram_tensor("cosk", [128, T], f32, kind="ExternalInput")
    sink_d = nc.dram_tensor("sink", [128, T], f32, kind="ExternalInput")
    trimask_d = nc.dram_tensor("trimask", [128, 128], f32, kind="ExternalInput")
    selmask_d = nc.dram_tensor("selmask", [128, NE], f32, kind="ExternalInput")
    ident_d = nc.dram_tensor("ident", [128, 128], f32r, kind="ExternalInput")
    ones_d = nc.dram_tensor("onescol", [128, 1], f32r, kind="ExternalInput")
    moeT_o = nc.dram_tensor("moeT", [HID, T], f32, kind="ExternalOutput")
    residT_o = nc.dram_tensor("residT", [HID, T], f32, kind="ExternalOutput")

    def r(ap):
        return ap.bitcast(f32r)

    def asf(ap):
        return ap.bitcast(f32)

    with tile.TileContext(nc) as tc, ExitStack() as top:
        cpool = top.enter_context(tc.tile_pool(name="consts", bufs=1))
        ident = cpool.tile([128, 128], f32r, tag="ident")
        nc.sync.dma_start(ident[:], ident_d[:])
        ones = cpool.tile([128, 1], f32r, tag="ones")
        nc.sync.dma_start(ones[:], ones_d[:])

        # DRAM bounce buffers for collectives (outputs Shared)
        dpool = top.enter_context(tc.tile_pool(name="dram", bufs=1,
                                               space="DRAM"))
        cout1 = nc.dram_tensor("cc_out1", [HID, T], f32r, addr_space="Shared")
        cout2 = nc.dram_tensor("cc_out2", [HID, T], f32, addr_space="Shared")

        attn_scope = ExitStack()
        apool = attn_scope.enter_context(tc.tile_pool(name="attn_act",
                                                      bufs=1))
        qhat = apool.tile([128, T], f32r, tag="qhat")
        khat = apool.tile([128, T], f32r, tag="khat")
        vT = apool.tile([128, T], f32r, tag="vT")

        def rmsnorm_scale(src_tiles, src_f32r, pool, pool1, pspool, tag):
            """src_tiles: 8 [128, T] tiles covering HID on partitions.
            Returns s [1, T] sbuf tile: rsqrt(mean_h(x^2) + eps)."""
            ps = pspool.tile([1, T], f32, tag=f"{tag}_ps")
            for ht in range(HT):
                src = src_tiles[ht][:]
                if src_f32r:
                    src = asf(src)
                for nt in range(T // 512):
                    sl = slice(nt * 512, nt * 512 + 512)
                    sq = pool.tile([128, 512], f32r, tag=f"{tag}_sq")
                    nc.scalar.square(sq[:], src[:, sl])
                    nc.tensor.matmul(ps[0:1, sl], ones[:], sq[:],
                                     start=(ht == 0), stop=(ht == HT - 1))
            epst = pool1.tile([1, 1], f32, tag=f"{tag}_eps")
            nc.gpsimd.memset(epst[:], EPS)
            srt = pool1.tile([1, T], f32, tag=f"{tag}_srt")
            nc.scalar.activation(srt[:], ps[0:1, :], AF.Sqrt,
                                 bias=epst[:], scale=1.0 / HID)
            s = pool1.tile([1, T], f32, tag=f"{tag}_s")
            nc.vector.reciprocal(s[:], srt[:])
            return s

        # ============ Phase 1: x1T = hiddenT * rsqrt(mean h^2+eps) =========
        with tc.tile_pool(name="p1", bufs=2) as p1, \
             tc.tile_pool(name="p1c", bufs=1) as p1c, \
             tc.tile_pool(name="p1ps", bufs=1, space="PSUM") as p1ps, \
             tc.tile_pool(name="p1x", bufs=1) as p1x, \
             tc.tile_pool(name="p2ps", bufs=1, space="PSUM") as p2ps:
            hts = []
            for ht in range(HT):
                t_ = p1x.tile([128, T], f32r, tag=f"ht{ht}")
                nc.sync.dma_start(t_[:], hT_d[ht * 128:(ht + 1) * 128, :])
                hts.append(t_)
            s1 = rmsnorm_scale(hts, True, p1, p1c, p1ps, "s1")
            S1 = p1x.tile([128, T], f32, tag="S1")
            nc.gpsimd.partition_broadcast(S1[:], s1[:])
            x1 = hts
            for ht in range(HT):
                # in-place normalize; output written as f32r
                nc.vector.tensor_tensor(x1[ht][:], asf(x1[ht][:]), S1[:],
                                        op=ALU.mult)

            # ============ Phase 2a: qkv + rope ============================
            qkvw = []
            for ht in range(HT):
                t_ = p1c.tile([128, 3 * HD], f32r, tag=f"qkvw{ht}")
                nc.sync.dma_start(t_[:], qkvT_d[ht * 128:(ht + 1) * 128, :])
                qkvw.append(t_)

            def load_rope(cd, sd):
                c_ = p1c.tile([128, T], f32, tag="rope_cos", name="rc")
                nc.sync.dma_start(c_[:], cd[:])
                s_ = p1c.tile([128, T], f32, tag="rope_sin", name="rs")
                nc.sync.dma_start(s_[:], sd[:])
                return c_, s_

            def qkv_mm(col, ps):
                for nt in range(T // 512):
                    sl = slice(nt * 512, nt * 512 + 512)
                    for ht in range(HT):
                        nc.tensor.matmul(
                            ps[:, sl],
                            qkvw[ht][:, col * 128:(col + 1) * 128],
                            x1[ht][:, sl],
                            start=(ht == 0), stop=(ht == HT - 1))

            def rope(ps, cos_t, sinsign_t, dst):
                # rope(x) = x*cos128 + swap_halves(x)*[-sin ; +sin]
                raw = p1c.tile([128, T], f32, tag="rope_raw", name="rr")
                nc.any.tensor_copy(raw[:], ps[:])
                sw = p1c.tile([128, T], f32, tag="rope_sw", name="rw")
                nc.sync.dma_start(sw[0:64, :], raw[64:128, :])
                nc.sync.dma_start(sw[64:128, :], raw[0:64, :])
                for nt in range(4):
                    sl = slice(nt * 512, nt * 512 + 512)
                    t1 = p1.tile([128, 512], f32, tag="rope_t1")
                    t2 = p1.tile([128, 512], f32, tag="rope_t2")
                    nc.vector.tensor_mul(t1[:], raw[:, sl], cos_t[:, sl])
                    nc.vector.tensor_mul(t2[:], sw[:, sl], sinsign_t[:, sl])
                    nc.vector.tensor_tensor(dst[:, sl], t1[:], t2[:],
                                            op=ALU.add)

            psq = p2ps.tile([128, T], f32, tag="qkv")
            qkv_mm(0, psq)
            cq_t, sq_t = load_rope(cosq_d, sinq_d)
            rope(psq, cq_t, sq_t, qhat)
            psk = p2ps.tile([128, T], f32, tag="qkv")
            qkv_mm(1, psk)
            ck_t, sk_t = load_rope(cosk_d, sink_d)
            rope(psk, ck_t, sk_t, khat)
            psv = p2ps.tile([128, T], f32, tag="qkv")
            qkv_mm(2, psv)
            nc.any.tensor_copy(vT[:], psv[:])

        # ============ Phase 2b: scores/softmax/PV =========================
        aop = attn_scope.enter_context(tc.tile_pool(name="aop", bufs=1))
        aoT = aop.tile([128, T], f32r, tag="aoT")
        with tc.tile_pool(name="p2b", bufs=2) as p2b, \
             tc.tile_pool(name="p2bx", bufs=1) as p2bx, \
             tc.tile_pool(name="sps", bufs=1, space="PSUM") as sps, \
             tc.tile_pool(name="tps", bufs=2, space="PSUM") as tps:
            trimask = p2bx.tile([128, 128], f32, tag="trimask")
            nc.sync.dma_start(trimask[:], trimask_d[:])
            vN = []
            for kt in range(TT):
                ps = tps.tile([128, 128], f32r, tag="pt_ps")
                nc.tensor.transpose(ps[:], vT[:, kt * 128:(kt + 1) * 128],
                                    ident[:])
                t_ = p2bx.tile([128, 128], f32r, tag=f"vN{kt}")
                nc.vector.tensor_copy(t_[:], ps[:])
                vN.append(t_)

            for qg in range(TT // 4):
                nkt = 4 * qg + 4
                PTs = [p2bx.tile([128, 512], f32r, tag=f"PT{kt}",
                                 name=f"PT{kt}_{qg}")
                       for kt in range(nkt)]
                rrow = p2b.tile([1, 512], f32, tag="rrow")
                for qi in range(4 * qg, 4 * qg + 4):
                    nk = (qi + 1) * 128
                    pss = sps.tile([128, T], f32, tag="scores")
                    for skx in range(0, nk, 512):
                        wk = min(512, nk - skx)
                        nc.tensor.matmul(
                            pss[:, skx:skx + wk],
                            qhat[:, qi * 128:(qi + 1) * 128],
                            khat[:, skx:skx + wk],
                            start=True, stop=True)
                    dsl = slice(qi * 128, (qi + 1) * 128)
                    nc.vector.tensor_add(pss[:, dsl], pss[:, dsl],
                                         trimask[:])
                    nm = p2b.tile([128, 1], f32, tag="nm")
                    nc.vector.reduce_max(nm[:], pss[:, 0:nk], axis=AX.X,
                                         negate=True)
                    probs = p2b.tile([128, T], f32r, tag="probs")
                    rsum = p2b.tile([128, 1], f32, tag="rsum")
                    nc.scalar.activation(probs[:, 0:nk], pss[:, 0:nk],
                                         AF.Exp, bias=nm[:], scale=1.0,
                                         accum_out=rsum[:])
                    rinv = p2b.tile([128, 1], f32, tag="rinv")
                    nc.vector.reciprocal(rinv[:], rsum[:])
                    rinv_r = p2b.tile([128, 1], f32r, tag="rinv_r")
                    nc.vector.tensor_copy(rinv_r[:], rinv[:])
                    pr = tps.tile([1, 128], f32r, tag="pt_ps")
                    nc.tensor.transpose(pr[:], rinv_r[:], ident[:])
                    nc.vector.tensor_copy(
                        asf(rrow[0:1, (qi % 4) * 128:(qi % 4 + 1) * 128]),
                        asf(pr[:]))
                    for kt in range(qi + 1):
                        pt = tps.tile([128, 128], f32r, tag="pt_ps")
                        nc.tensor.transpose(
                            pt[:], probs[:, kt * 128:(kt + 1) * 128],
                            ident[:])
                        nc.vector.tensor_copy(
                            PTs[kt][:, (qi % 4) * 128:(qi % 4 + 1) * 128],
                            pt[:])
                RB = p2b.tile([128, 512], f32, tag="RB")
                nc.gpsimd.partition_broadcast(RB[:], rrow[:])
                pso = tps.tile([128, 512], f32, tag="av")
                for kt in range(nkt):
                    c0 = max(0, (kt - 4 * qg) * 128)
                    nc.tensor.matmul(pso[:, c0:512], vN[kt][:],
                                     PTs[kt][:, c0:512],
                                     start=(kt == 0), stop=(kt == nkt - 1),
                                     skip_group_check=True)
                nc.vector.tensor_tensor(aoT[:, qg * 512:(qg + 1) * 512],
                                        pso[:], RB[:], op=ALU.mult)

        # ============ Phase 2c: AllGather heads + o_proj + residual =======
        with tc.tile_pool(name="p2c", bufs=1) as p2c, \
             tc.tile_pool(name="ops", bufs=1, space="PSUM") as ops:
            cin1 = dpool.tile([128, T], f32r, tag="cin1")
            nc.sync.dma_start(cin1[:], aoT[:])
            nc.gpsimd.collective_compute(
                "AllGather", mybir.AluOpType.bypass,
                replica_groups=[list(range(N_CORES))],
                ins=[cin1.opt()], outs=[cout1.ap()])
            aof = []
            for rt in range(HT):
                t_ = p2c.tile([128, T], f32r, tag=f"aof{rt}")
                nc.sync.dma_start(t_[:], cout1[rt * 128:(rt + 1) * 128, :])
                aof.append(t_)
            oww = []
            for rt in range(HT):
                t_ = p2c.tile([128, 128], f32r, tag=f"oww{rt}")
                nc.sync.dma_start(t_[:], owT_d[rt * 128:(rt + 1) * 128, :])
                oww.append(t_)
            pso = ops.tile([128, T], f32, tag="o")
            for nt in range(T // 512):
                sl = slice(nt * 512, nt * 512 + 512)
                for rt in range(HT):
                    nc.tensor.matmul(pso[:, sl], oww[rt][:],
                                     aof[rt][:, sl],
                                     start=(rt == 0), stop=(rt == HT - 1))
            hsl = p2c.tile([128, T], f32, tag="hsl")
            nc.sync.dma_start(hsl[:], hslice_d[:])
            rsl = p2c.tile([128, T], f32, tag="rsl")
            nc.vector.tensor_add(rsl[:], pso[:], hsl[:])
            cin2 = dpool.tile([128, T], f32, tag="cin2")
            nc.sync.dma_start(cin2[:], rsl[:])
            nc.gpsimd.collective_compute(
                "AllGather", mybir.AluOpType.bypass,
                replica_groups=[list(range(N_CORES))],
                ins=[cin2.opt()], outs=[cout2.ap()])
            nc.sync.dma_start(residT_o[:], cout2.ap())

        attn_scope.close()

        # ============ Phase 3: x2T + fp32 router logits ===================
        x2pool = top.enter_context(tc.tile_pool(name="x2", bufs=1))
        lgpool = top.enter_context(tc.tile_pool(name="lgp", bufs=1))
        lgT = lgpool.tile([NE, T], f32, tag="lgT")
        x2 = []
        with tc.tile_pool(name="p3", bufs=2) as p3, \
             tc.tile_pool(name="p3x", bufs=1) as p3x, \
             tc.tile_pool(name="p3ps", bufs=1, space="PSUM") as p3ps, \
             tc.tile_pool(name="p3lps", bufs=1, space="PSUM") as p3lps:
            rts = []
            for ht in range(HT):
                t_ = p3x.tile([128, T], f32, tag=f"rt{ht}")
                nc.sync.dma_start(t_[:], cout2[ht * 128:(ht + 1) * 128, :])
                rts.append(t_)
            s2 = rmsnorm_scale(rts, False, p3, p3x, p3ps, "s2")
            S2 = p3x.tile([128, T], f32, tag="S2")
            nc.gpsimd.partition_broadcast(S2[:], s2[:])
            # router logits in full fp32 on the *unnormalized* residual
            # (the rmsnorm scale s2[t] commutes with the h-contraction)
            gww = []
            for ht in range(HT):
                t_ = p3x.tile([128, NE], f32, tag=f"gww{ht}")
                nc.sync.dma_start(t_[:], gwT_d[ht * 128:(ht + 1) * 128, :])
                gww.append(t_)
            psl = p3lps.tile([NE, T], f32, tag="lg")
            for nt in range(T // 512):
                sl = slice(nt * 512, nt * 512 + 512)
                for ht in range(HT):
                    nc.tensor.matmul(psl[:, sl], gww[ht][:],
                                     rts[ht][:, sl],
                                     start=(ht == 0), stop=(ht == HT - 1))
            s2b8 = p3x.tile([NE, T], f32, tag="s2b8")
            nc.gpsimd.partition_broadcast(s2b8[:], s2[:])
            nc.vector.tensor_tensor(lgT[:], psl[:], s2b8[:], op=ALU.mult)
            for ht in range(HT):
                t_ = x2pool.tile([128, T], f32r, tag=f"x2{ht}")
                nc.vector.tensor_tensor(t_[:], rts[ht][:], S2[:],
                                        op=ALU.mult)
                x2.append(t_)

        # ============ Phase 4: router -> CW [128, T] ======================
        cwpool = top.enter_context(tc.tile_pool(name="cw", bufs=1))
        CW = cwpool.tile([128, T], f32, tag="CW")
        with tc.tile_pool(name="p4", bufs=2) as p4, \
             tc.tile_pool(name="p4x", bufs=1) as p4x, \
             tc.tile_pool(name="p4tps", bufs=2, space="PSUM") as p4tps:
            selm = p4x.tile([128, NE], f32, tag="selm")
            nc.sync.dma_start(selm[:], selmask_d[:])
            cwrow = p4x.tile([1, T], f32, tag="cwrow")
            for tt in range(TT):
                tsl = slice(tt * 128, (tt + 1) * 128)
                p8 = p4tps.tile([128, NE], f32, tag="lgt_ps")
                nc.tensor.transpose(p8[:], lgT[:, tsl],
                                    asf(ident[0:NE, 0:NE]))
                lgN = p4.tile([128, NE], f32, tag="lgN")
                nc.vector.tensor_copy(lgN[:], p8[:])
                m1 = p4.tile([128, 1], f32, tag="m1")
                nc.vector.reduce_max(m1[:], lgN[:], axis=AX.X)
                eq = p4.tile([128, NE], f32, tag="eq")
                nc.vector.tensor_scalar(eq[:], lgN[:], m1[:], None,
                                        op0=ALU.is_equal)
                msk = p4.tile([128, NE], f32, tag="msk")
                nc.vector.scalar_tensor_tensor(
                    msk[:], eq[:], -1e30, lgN[:],
                    op0=ALU.mult, op1=ALU.add)
                m2 = p4.tile([128, 1], f32, tag="m2")
                nc.vector.reduce_max(m2[:], msk[:], axis=AX.X)
                fl = p4.tile([128, NE], f32, tag="fl")
                nc.vector.tensor_scalar(fl[:], lgN[:], m2[:], None,
                                        op0=ALU.is_ge)
                dd = p4.tile([128, NE], f32, tag="dd")
                nc.vector.tensor_scalar(dd[:], lgN[:], m1[:], None,
                                        op0=ALU.subtract)
                e1 = p4.tile([128, NE], f32, tag="e1")
                nc.scalar.activation(e1[:], dd[:], AF.Exp)
                se = p4.tile([128, NE], f32, tag="se")
                den = p4.tile([128, 1], f32, tag="den")
                nc.vector.scalar_tensor_tensor(se[:], fl[:], 1.0, e1[:],
                                               op0=ALU.mult, op1=ALU.mult,
                                               accum_out=den[:])
                rv = p4.tile([128, 1], f32, tag="rv")
                nc.vector.reciprocal(rv[:], den[:])
                csel = p4.tile([128, NE], f32, tag="csel")
                cws = p4.tile([128, 1], f32, tag="cws")
                nc.vector.scalar_tensor_tensor(csel[:], se[:], 1.0, selm[:],
                                               op0=ALU.mult, op1=ALU.mult,
                                               accum_out=cws[:])
                cwn = p4.tile([128, 1], f32r, tag="cwn")
                nc.vector.tensor_tensor(cwn[:], cws[:], rv[:], op=ALU.mult)
                pc = p4tps.tile([1, 128], f32r, tag="cw_ps")
                nc.tensor.transpose(pc[:], cwn[:], ident[:])
                nc.vector.tensor_copy(asf(cwrow[0:1, tsl]), asf(pc[:]))
            nc.gpsimd.partition_broadcast(CW[:], cwrow[:])

        # ============ Phase 5: expert FFN =================================
        with tc.tile_pool(name="p5w", bufs=2) as p5w, \
             tc.tile_pool(name="p5h", bufs=1) as p5h, \
             tc.tile_pool(name="p5s", bufs=2) as p5s, \
             tc.tile_pool(name="p5ps", bufs=2, space="PSUM") as p5ps:
            for ts in range(NTS):
                sl = slice(ts * TSL, (ts + 1) * TSL)
                hts_ = []
                for it in range(ITI):
                    w1b = p5w.tile([128, HT, 128], f32r, tag="w1b")
                    nc.sync.dma_start(w1b[:], w1T_d[it])
                    w3b = p5w.tile([128, HT, 128], f32r, tag="w3b")
                    nc.sync.dma_start(w3b[:], w3T_d[it])
                    pg = p5ps.tile([128, TSL], f32, tag="g")
                    pu = p5ps.tile([128, TSL], f32, tag="u")
                    for ht in range(HT):
                        nc.tensor.matmul(pg[:], w1b[:, ht, :],
                                         x2[ht][:, sl],
                                         start=(ht == 0),
                                         stop=(ht == HT - 1))
                    for ht in range(HT):
                        nc.tensor.matmul(pu[:], w3b[:, ht, :],
                                         x2[ht][:, sl],
                                         start=(ht == 0),
                                         stop=(ht == HT - 1))
                    sg = p5s.tile([128, TSL], f32, tag="sg")
                    nc.scalar.activation(sg[:], pg[:], AF.Silu)
                    ht_ = p5h.tile([128, TSL], f32r, tag=f"h{it}")
                    nc.vector.tensor_tensor(ht_[:], sg[:], pu[:],
                                            op=ALU.mult)
                    hts_.append(ht_)
                for dtt in range(HT):
                    w2b = p5w.tile([128, ITI, 128], f32r, tag="w2b")
                    nc.sync.dma_start(w2b[:], w2T_d[dtt])
                    pd = p5ps.tile([128, TSL], f32, tag="d")
                    for it in range(ITI):
                        nc.tensor.matmul(pd[:], w2b[:, it, :],
                                         hts_[it][:],
                                         start=(it == 0),
                                         stop=(it == ITI - 1))
                    mo = p5s.tile([128, TSL], f32, tag="mo")
                    nc.vector.tensor_tensor(mo[:], pd[:], CW[:, sl],
                                            op=ALU.mult)
                    nc.sync.dma_start(
                        moeT_o[dtt * 128:(dtt + 1) * 128, sl], mo[:])

    nc.compile()
    return nc


def _host_prep(positions, hidden_states, qkv_w, o_w, gate_w, w1, w2, w3,
               ln1_w, ln2_w):
    f = np.float32
    pos = np.asarray(positions).astype(f)
    hidden = np.asarray(hidden_states, dtype=f)
    hT = np.ascontiguousarray(hidden.T)

    half = HD // 2
    inv = THETA ** (-np.arange(half, dtype=f) * 2.0 / HD)
    ang = inv[:, None] * pos[None, :]          # [64, T]
    cos = np.cos(ang).astype(f)
    sin = np.sin(ang).astype(f)
    cos128 = np.concatenate([cos, cos], axis=0)
    # rope(x) = x*cos128 + swap_halves(x)*sinsign, sinsign = [-sin ; +sin]
    sinsign = np.concatenate([-sin, sin], axis=0)
    cosq, sinq = (cos128 * SCALE).astype(f), (sinsign * SCALE).astype(f)
    cosk, sink = cos128.astype(f), sinsign.astype(f)

    qq, kk = np.meshgrid(np.arange(128), np.arange(128), indexing="ij")
    trimask = np.where(kk <= qq, 0.0, -1e30).astype(f)
    ident = np.eye(128, dtype=f)
    ones = np.ones((128, 1), dtype=f)

    qkv_f = (np.asarray(qkv_w, dtype=f) * np.asarray(ln1_w, dtype=f)[None, :])
    gate_f = (np.asarray(gate_w, dtype=f) * np.asarray(ln2_w, dtype=f)[None, :])
    gwT = np.ascontiguousarray(gate_f.T)       # [HID, 8]
    ln2 = np.asarray(ln2_w, dtype=f)

    in_maps = []
    for c in range(N_CORES):
        kvh = c // (NH // NKV)
        qs = qkv_f[c * HD:(c + 1) * HD]                       # [128, HID]
        ks = qkv_f[NH * HD + kvh * HD: NH * HD + (kvh + 1) * HD]
        vs = qkv_f[(NH + NKV) * HD + kvh * HD:
                   (NH + NKV) * HD + (kvh + 1) * HD]
        qkvT = np.ascontiguousarray(
            np.concatenate([qs, ks, vs], axis=0).T)           # [HID, 384]
        owT = np.ascontiguousarray(
            np.asarray(o_w, dtype=f)[c * 128:(c + 1) * 128, :].T)  # [HID,128]
        w1c = np.asarray(w1[c], dtype=f) * ln2[None, :]       # [INTER, HID]
        w3c = np.asarray(w3[c], dtype=f) * ln2[None, :]
        w2c = np.asarray(w2[c], dtype=f)                      # [HID, INTER]
        # [ITI, 128p, HT, 128f]: lhsT tile [p, a, f] loads contiguously
        w1T = np.ascontiguousarray(
            w1c.T.reshape(HT, 128, ITI, 128).transpose(2, 1, 0, 3))
        w3T = np.ascontiguousarray(
            w3c.T.reshape(HT, 128, ITI, 128).transpose(2, 1, 0, 3))
        w2T = np.ascontiguousarray(
            w2c.T.reshape(ITI, 128, HT, 128).transpose(2, 1, 0, 3))
        selmask = np.zeros((128, NE), dtype=f)
        selmask[:, c] = 1.0
        in_maps.append({
            "hT": hT,
            "hslice": np.ascontiguousarray(hT[c * 128:(c + 1) * 128]),
            "qkvT": qkvT, "owT": owT, "gwT": gwT,
            "w1T": w1T, "w3T": w3T, "w2T": w2T,
            "cosq": cosq, "sinq": sinq, "cosk": cosk, "sink": sink,
            "trimask": trimask, "selmask": selmask, "ident": ident,
            "onescol": ones,
        })
    return in_maps


def kernel(positions, hidden_states, qkv_w, o_w, gate_w, w1, w2, w3,
           ln1_w, ln2_w, _trace=False):
    from concourse.bass_utils import run_bass_kernel_spmd
    if "nc" not in _CACHE:
        _CACHE["nc"] = _build_nc()
    nc = _CACHE["nc"]
    in_maps = _host_prep(positions, hidden_states, qkv_w, o_w, gate_w,
                         w1, w2, w3, ln1_w, ln2_w)
    res = run_bass_kernel_spmd(nc, in_maps, list(range(N_CORES)),
                               trace=_trace)
    _CACHE["last_result"] = res
    moeT = np.zeros((HID, T), dtype=np.float64)
    for c in range(N_CORES):
        moeT += res.results[c]["moeT"]
    moe = np.ascontiguousarray(moeT.T).astype(np.float32)
    resid = np.ascontiguousarray(res.results[0]["residT"].T)
    return (moe, resid)


# revision 27
# speedup vs baseline: 1.0731x; 1.0068x over previous
"""Mixtral decoder layer (attention + top-2-of-8 MoE) on 8 trn2 NeuronCores.

Sharding: attention is head-parallel (8 heads -> 1 head/core, GQA kv head =
core//4), o_proj is sharded over output rows; two 1MB AllGathers knit the
cores back together.  The MoE is expert-parallel (8 experts -> 1 expert/core);
each core computes its expert's contribution weighted by the dense top-2
router weight and the host sums the 8 partial outputs.

Device layout is transposed throughout: activations live as [feature, token]
so every matmul contraction dim sits on the SBUF partition axis.  The host
pre-transposes weights (and folds the rmsnorm gains into the adjacent weight
matrices) and un-transposes the outputs.

All matmuls run in float32r (full-rate).  Walrus requires every operand of an
fp32r matmul to be *produced* as fp32r, so matmul-feeding tiles are declared
float32r and written through f32r APs; non-matmul consumers read them through
.bitcast(float32).
"""

import numpy as np

T, HID, NH, NKV, HD = 2048, 1024, 8, 2, 128
INTER, NE, TOPK = 3584, 8, 2
EPS, THETA = 1e-5, 10000.0
N_CORES = 8
HT = HID // 128    # 8 h-tiles
TT = T // 128      # 16 t-tiles
ITI = INTER // 128  # 28 i-tiles
TSL = 512          # token-slice for the FFN phase
NTS = T // TSL
SCALE = HD ** -0.5

_CACHE = {}


def _build_nc():
    import concourse.bacc as bacc
    import concourse.tile as tile
    import concourse.mybir as mybir
    from contextlib import ExitStack

    dt = mybir.dt
    f32 = dt.float32
    f32r = dt.float32r
    AF = mybir.ActivationFunctionType
    ALU = mybir.AluOpType
    AX = mybir.AxisListType

    nc = bacc.Bacc("TRN2", target_bir_lowering=False, debug=False,
                   num_devices=N_CORES)

    # ---- DRAM I/O ----  (tensors feeding matmuls are float32r: same bits)
    hT_d = nc.dram_tensor("hT", [HID, T], f32r, kind="ExternalInput")
    hslice_d = nc.dram_tensor("hslice", [128, T], f32, kind="ExternalInput")
    qkvT_d = nc.dram_tensor("qkvT", [HID, 3 * HD], f32r, kind="ExternalInput")
    owT_d = nc.dram_tensor("owT", [HID, 128], f32r, kind="ExternalInput")
    gwT_d = nc.dram_tensor("gwT", [HID, NE], f32, kind="ExternalInput")
    w1T_d = nc.dram_tensor("w1T", [ITI, 128, HT, 128], f32r,
                           kind="ExternalInput")
    w3T_d = nc.dram_tensor("w3T", [ITI, 128, HT, 128], f32r,
                           kind="ExternalInput")
    w2T_d = nc.dram_tensor("w2T", [HT, 128, ITI, 128], f32r,
                           kind="ExternalInput")
    cosq_d = nc.dram_tensor("cosq", [128, T], f32, kind="ExternalInput")
    sinq_d = nc.dram_tensor("sinq", [128, T], f32, kind="ExternalInput")
    cosk_d = nc.dram_tensor("cosk", [128, T], f32, kind="ExternalInput")
    sink_d = nc.dram_tensor("sink", [128, T], f32, kind="ExternalInput")
    trimask_d = nc.dram_tensor("trimask", [128, 128], f32, kind="ExternalInput")
    selmask_d = nc.dram_tensor("selmask", [128, NE], f32, kind="ExternalInput")
    ident_d = nc.dram_tensor("ident", [128, 128], f32r, kind="ExternalInput")
    ones_d = nc.dram_tensor("onescol", [128, 1], f32r, kind="ExternalInput")
    moeT_o = nc.dram_tensor("moeT", [HID, T], f32, kind="ExternalOutput")
    residT_o = nc.dram_tensor("residT", [HID, T], f32, kind="ExternalOutput")

    def r(ap):
        return ap.bitcast(f32r)

    def asf(ap):
        return ap.bitcast(f32)

    with tile.TileContext(nc) as tc, ExitStack() as top:
        cpool = top.enter_context(tc.tile_pool(name="consts", bufs=1))
        ident = cpool.tile([128, 128], f32r, tag="ident")
        nc.sync.dma_start(ident[:], ident_d[:])
        ones = cpool.tile([128, 1], f32r, tag="ones")
        nc.sync.dma_start(ones[:], ones_d[:])

        # DRAM bounce buffers for collectives (outputs Shared)
        dpool = top.enter_context(tc.tile_pool(name="dram", bufs=1,
                                               space="DRAM"))
        cout1 = nc.dram_tensor("cc_out1", [HID, T], f32r, addr_space="Shared")
        cout2 = nc.dram_tensor("cc_out2", [HID, T], f32, addr_space="Shared")

        attn_scope = ExitStack()
        apool = attn_scope.enter_context(tc.tile_pool(name="attn_act",
                                                      bufs=1))
        qhat = apool.tile([128, T], f32r, tag="qhat")
        khat = apool.tile([128, T], f32r, tag="khat")
        vT = apool.tile([128, T], f32r, tag="vT")

        def rmsnorm_scale(src_tiles, src_f32r, pool, pool1, pspool, tag):
            """src_tiles: 8 [128, T] tiles covering HID on partitions.
            Returns s [1, T] sbuf tile: rsqrt(mean_h(x^2) + eps)."""
            ps = pspool.tile([1, T], f32, tag=f"{tag}_ps")
            for ht in range(HT):
                src = src_tiles[ht][:]
                if src_f32r:
                    src = asf(src)
                for nt in range(T // 512):
                    sl = slice(nt * 512, nt * 512 + 512)
                    sq = pool.tile([128, 512], f32r, tag=f"{tag}_sq")
                    nc.scalar.square(sq[:], src[:, sl])
                    nc.tensor.matmul(ps[0:1, sl], ones[:], sq[:],
                                     start=(ht == 0), stop=(ht == HT - 1))
            epst = pool1.tile([1, 1], f32, tag=f"{tag}_eps")
            nc.gpsimd.memset(epst[:], EPS)
            srt = pool1.tile([1, T], f32, tag=f"{tag}_srt")
            nc.scalar.activation(srt[:], ps[0:1, :], AF.Sqrt,
                                 bias=epst[:], scale=1.0 / HID)
            s = pool1.tile([1, T], f32, tag=f"{tag}_s")
            nc.vector.reciprocal(s[:], srt[:])
            return s

        # ============ Phase 1: x1T = hiddenT * rsqrt(mean h^2+eps) =========
        with tc.tile_pool(name="p1", bufs=2) as p1, \
             tc.tile_pool(name="p1c", bufs=1) as p1c, \
             tc.tile_pool(name="p1x", bufs=1) as p1x:
            hts = []
            for ht in range(HT):
                t_ = p1x.tile([128, T], f32r, tag=f"ht{ht}")
                nc.sync.dma_start(t_[:], hT_d[ht * 128:(ht + 1) * 128, :])
                hts.append(t_)
            with tc.tile_pool(name="p1ps", bufs=1, space="PSUM") as p1ps:
                s1 = rmsnorm_scale(hts, True, p1, p1c, p1ps, "s1")
            p2ps_ctx = tc.tile_pool(name="p2ps", bufs=2, space="PSUM")
            p2ps = p2ps_ctx.__enter__()
            S1 = p1x.tile([128, T], f32, tag="S1")
            nc.gpsimd.partition_broadcast(S1[:], s1[:])
            x1 = hts
            for ht in range(HT):
                # in-place normalize; output written as f32r
                nc.vector.tensor_tensor(x1[ht][:], asf(x1[ht][:]), S1[:],
                                        op=ALU.mult)

            # ============ Phase 2a: qkv + rope ============================
            qkvw = []
            for ht in range(HT):
                t_ = p1c.tile([128, 3 * HD], f32r, tag=f"qkvw{ht}")
                nc.sync.dma_start(t_[:], qkvT_d[ht * 128:(ht + 1) * 128, :])
                qkvw.append(t_)

            def load_rope(cd, sd):
                c_ = p1c.tile([128, T], f32, tag="rope_cos", name="rc")
                nc.sync.dma_start(c_[:], cd[:])
                s_ = p1c.tile([128, T], f32, tag="rope_sin", name="rs")
                nc.sync.dma_start(s_[:], sd[:])
                return c_, s_

            def qkv_mm(col, ps):
                for nt in range(T // 512):
                    sl = slice(nt * 512, nt * 512 + 512)
                    for ht in range(HT):
                        nc.tensor.matmul(
                            ps[:, sl],
                            qkvw[ht][:, col * 128:(col + 1) * 128],
                            x1[ht][:, sl],
                            start=(ht == 0), stop=(ht == HT - 1))

            def rope(ps, cos_t, sinsign_t, dst):
                # rope(x) = x*cos128 + swap_halves(x)*[-sin ; +sin]
                raw = p1c.tile([128, T], f32, tag="rope_raw", name="rr")
                nc.any.tensor_copy(raw[:], ps[:])
                sw = p1c.tile([128, T], f32, tag="rope_sw", name="rw")
                nc.sync.dma_start(sw[0:64, :], raw[64:128, :])
                nc.sync.dma_start(sw[64:128, :], raw[0:64, :])
                for nt in range(4):
                    sl = slice(nt * 512, nt * 512 + 512)
                    t1 = p1.tile([128, 512], f32, tag="rope_t1")
                    t2 = p1.tile([128, 512], f32, tag="rope_t2")
                    nc.vector.tensor_mul(t1[:], raw[:, sl], cos_t[:, sl])
                    nc.vector.tensor_mul(t2[:], sw[:, sl], sinsign_t[:, sl])
                    nc.vector.tensor_tensor(dst[:, sl], t1[:], t2[:],
                                            op=ALU.add)

            psq = p2ps.tile([128, T], f32, tag="qkv")
            qkv_mm(0, psq)
            cq_t, sq_t = load_rope(cosq_d, sinq_d)
            rope(psq, cq_t, sq_t, qhat)
            psk = p2ps.tile([128, T], f32, tag="qkv")
            qkv_mm(1, psk)
            ck_t, sk_t = load_rope(cosk_d, sink_d)
            rope(psk, ck_t, sk_t, khat)
            psv = p2ps.tile([128, T], f32, tag="qkv")
            qkv_mm(2, psv)
            nc.any.tensor_copy(vT[:], psv[:])
            p2ps_ctx.__exit__(None, None, None)

        # ============ Phase 2b: scores/softmax/PV =========================
        aop = attn_scope.enter_context(tc.tile_pool(name="aop", bufs=1))
        aoT = aop.tile([128, T], f32r, tag="aoT")
        with tc.tile_pool(name="p2b", bufs=2) as p2b, \
             tc.tile_pool(name="p2bx", bufs=1) as p2bx, \
             tc.tile_pool(name="sps", bufs=1, space="PSUM") as sps, \
             tc.tile_pool(name="tps", bufs=2, space="PSUM") as tps:
            trimask = p2bx.tile([128, 128], f32, tag="trimask")
            nc.sync.dma_start(trimask[:], trimask_d[:])
            vN = []
            for kt in range(TT):
                ps = tps.tile([128, 128], f32r, tag="pt_ps")
                nc.tensor.transpose(ps[:], vT[:, kt * 128:(kt + 1) * 128],
                                    ident[:])
                t_ = p2bx.tile([128, 128], f32r, tag=f"vN{kt}")
                nc.vector.tensor_copy(t_[:], ps[:])
                vN.append(t_)

            for qg in range(TT // 4):
                nkt = 4 * qg + 4
                PTs = [p2bx.tile([128, 512], f32r, tag=f"PT{kt}",
                                 name=f"PT{kt}_{qg}")
                       for kt in range(nkt)]
                rrow = p2b.tile([1, 512], f32, tag="rrow")
                for qi in range(4 * qg, 4 * qg + 4):
                    nk = (qi + 1) * 128
                    pss = sps.tile([128, T], f32, tag="scores")
                    for skx in range(0, nk, 512):
                        wk = min(512, nk - skx)
                        nc.tensor.matmul(
                            pss[:, skx:skx + wk],
                            qhat[:, qi * 128:(qi + 1) * 128],
                            khat[:, skx:skx + wk],
                            start=True, stop=True)
                    dsl = slice(qi * 128, (qi + 1) * 128)
                    nc.vector.tensor_add(pss[:, dsl], pss[:, dsl],
                                         trimask[:])
                    nm = p2b.tile([128, 1], f32, tag="nm")
                    nc.vector.reduce_max(nm[:], pss[:, 0:nk], axis=AX.X,
                                         negate=True)
                    probs = p2b.tile([128, T], f32r, tag="probs")
                    rsum = p2b.tile([128, 1], f32, tag="rsum")
                    nc.scalar.activation(probs[:, 0:nk], pss[:, 0:nk],
                                         AF.Exp, bias=nm[:], scale=1.0,
                                         accum_out=rsum[:])
                    rinv = p2b.tile([128, 1], f32, tag="rinv")
                    nc.vector.reciprocal(rinv[:], rsum[:])
                    rinv_r = p2b.tile([128, 1], f32r, tag="rinv_r")
                    nc.vector.tensor_copy(rinv_r[:], rinv[:])
                    pr = tps.tile([1, 128], f32r, tag="pt_ps")
                    nc.tensor.transpose(pr[:], rinv_r[:], ident[:])
                    nc.vector.tensor_copy(
                        asf(rrow[0:1, (qi % 4) * 128:(qi % 4 + 1) * 128]),
                        asf(pr[:]))
                    for kt in range(qi + 1):
                        pt = tps.tile([128, 128], f32r, tag="pt_ps")
                        nc.tensor.transpose(
                            pt[:], probs[:, kt * 128:(kt + 1) * 128],
                            ident[:])
                        nc.vector.tensor_copy(
                            PTs[kt][:, (qi % 4) * 128:(qi % 4 + 1) * 128],
                            pt[:])
                RB = p2b.tile([128, 512], f32, tag="RB")
                nc.gpsimd.partition_broadcast(RB[:], rrow[:])
                pso = tps.tile([128, 512], f32, tag="av")
                for kt in range(nkt):
                    c0 = max(0, (kt - 4 * qg) * 128)
                    nc.tensor.matmul(pso[:, c0:512], vN[kt][:],
                                     PTs[kt][:, c0:512],
                                     start=(kt == 0), stop=(kt == nkt - 1),
                                     skip_group_check=True)
                nc.vector.tensor_tensor(aoT[:, qg * 512:(qg + 1) * 512],
                                        pso[:], RB[:], op=ALU.mult)

        # ============ Phase 2c: AllGather heads + o_proj + residual =======
        with tc.tile_pool(name="p2c", bufs=1) as p2c, \
             tc.tile_pool(name="ops", bufs=1, space="PSUM") as ops:
            cin1 = dpool.tile([128, T], f32r, tag="cin1")
            nc.sync.dma_start(cin1[:], aoT[:])
            nc.gpsimd.collective_compute(
                "AllGather", mybir.AluOpType.bypass,
                replica_groups=[list(range(N_CORES))],
                ins=[cin1.opt()], outs=[cout1.ap()])
            aof = []
            for rt in range(HT):
                t_ = p2c.tile([128, T], f32r, tag=f"aof{rt}")
                nc.sync.dma_start(t_[:], cout1[rt * 128:(rt + 1) * 128, :])
                aof.append(t_)
            oww = []
            for rt in range(HT):
                t_ = p2c.tile([128, 128], f32r, tag=f"oww{rt}")
                nc.sync.dma_start(t_[:], owT_d[rt * 128:(rt + 1) * 128, :])
                oww.append(t_)
            pso = ops.tile([128, T], f32, tag="o")
            for nt in range(T // 512):
                sl = slice(nt * 512, nt * 512 + 512)
                for rt in range(HT):
                    nc.tensor.matmul(pso[:, sl], oww[rt][:],
                                     aof[rt][:, sl],
                                     start=(rt == 0), stop=(rt == HT - 1))
            hsl = p2c.tile([128, T], f32, tag="hsl")
            nc.sync.dma_start(hsl[:], hslice_d[:])
            rsl = p2c.tile([128, T], f32, tag="rsl")
            nc.vector.tensor_add(rsl[:], pso[:], hsl[:])
            cin2 = dpool.tile([128, T], f32, tag="cin2")
            nc.sync.dma_start(cin2[:], rsl[:])
            nc.gpsimd.collective_compute(
                "AllGather", mybir.AluOpType.bypass,
                replica_groups=[list(range(N_CORES))],
                ins=[cin2.opt()], outs=[cout2.ap()])
            nc.sync.dma_start(residT_o[:], cout2.ap())

        attn_scope.close()

        # ============ Phase 3: x2T + fp32 router logits ===================
        x2pool = top.enter_context(tc.tile_pool(name="x2", bufs=1))
        cwpool = top.enter_context(tc.tile_pool(name="cw", bufs=1))
        CW = cwpool.tile([128, T], f32, tag="CW")
        lg_scope = ExitStack()
        lgpool = lg_scope.enter_context(tc.tile_pool(name="lgp", bufs=1))
        lgT = lgpool.tile([NE, T], f32, tag="lgT")
        x2 = []
        with tc.tile_pool(name="p3", bufs=2) as p3, \
             tc.tile_pool(name="p3x", bufs=1) as p3x, \
             tc.tile_pool(name="p3ps", bufs=1, space="PSUM") as p3ps, \
             tc.tile_pool(name="p3lps", bufs=1, space="PSUM") as p3lps:
            rts = []
            for ht in range(HT):
                t_ = p3x.tile([128, T], f32, tag=f"rt{ht}")
                nc.sync.dma_start(t_[:], cout2[ht * 128:(ht + 1) * 128, :])
                rts.append(t_)
            s2 = rmsnorm_scale(rts, False, p3, p3x, p3ps, "s2")
            S2 = p3x.tile([128, T], f32, tag="S2")
            nc.gpsimd.partition_broadcast(S2[:], s2[:])
            # router logits in full fp32 on the *unnormalized* residual
            # (the rmsnorm scale s2[t] commutes with the h-contraction)
            gww = []
            for ht in range(HT):
                t_ = p3x.tile([128, NE], f32, tag=f"gww{ht}")
                nc.sync.dma_start(t_[:], gwT_d[ht * 128:(ht + 1) * 128, :])
                gww.append(t_)
            psl = p3lps.tile([NE, T], f32, tag="lg")
            for nt in range(T // 512):
                sl = slice(nt * 512, nt * 512 + 512)
                for ht in range(HT):
                    nc.tensor.matmul(psl[:, sl], gww[ht][:],
                                     rts[ht][:, sl],
                                     start=(ht == 0), stop=(ht == HT - 1))
            s2b8 = p3x.tile([NE, T], f32, tag="s2b8")
            nc.gpsimd.partition_broadcast(s2b8[:], s2[:])
            nc.vector.tensor_tensor(lgT[:], psl[:], s2b8[:], op=ALU.mult)
            for ht in range(HT):
                t_ = x2pool.tile([128, T], f32r, tag=f"x2{ht}")
                nc.vector.tensor_tensor(t_[:], rts[ht][:], S2[:],
                                        op=ALU.mult)
                x2.append(t_)

        # ============ Phase 4: router -> CW [128, T] ======================
        with tc.tile_pool(name="p4", bufs=2) as p4, \
             tc.tile_pool(name="p4x", bufs=1) as p4x, \
             tc.tile_pool(name="p4tps", bufs=2, space="PSUM") as p4tps:
            selm = p4x.tile([128, NE], f32, tag="selm")
            nc.sync.dma_start(selm[:], selmask_d[:])
            cwrow = p4x.tile([1, T], f32, tag="cwrow")
            for tt in range(TT):
                tsl = slice(tt * 128, (tt + 1) * 128)
                p8 = p4tps.tile([128, NE], f32, tag="lgt_ps")
                nc.tensor.transpose(p8[:], lgT[:, tsl],
                                    asf(ident[0:NE, 0:NE]))
                lgN = p4.tile([128, NE], f32, tag="lgN")
                nc.vector.tensor_copy(lgN[:], p8[:])
                m1 = p4.tile([128, 1], f32, tag="m1")
                nc.vector.reduce_max(m1[:], lgN[:], axis=AX.X)
                eq = p4.tile([128, NE], f32, tag="eq")
                nc.vector.tensor_scalar(eq[:], lgN[:], m1[:], None,
                                        op0=ALU.is_equal)
                msk = p4.tile([128, NE], f32, tag="msk")
                nc.vector.scalar_tensor_tensor(
                    msk[:], eq[:], -1e30, lgN[:],
                    op0=ALU.mult, op1=ALU.add)
                m2 = p4.tile([128, 1], f32, tag="m2")
                nc.vector.reduce_max(m2[:], msk[:], axis=AX.X)
                fl = p4.tile([128, NE], f32, tag="fl")
                nc.vector.tensor_scalar(fl[:], lgN[:], m2[:], None,
                                        op0=ALU.is_ge)
                dd = p4.tile([128, NE], f32, tag="dd")
                nc.vector.tensor_scalar(dd[:], lgN[:], m1[:], None,
                                        op0=ALU.subtract)
                e1 = p4.tile([128, NE], f32, tag="e1")
                nc.scalar.activation(e1[:], dd[:], AF.Exp)
                se = p4.tile([128, NE], f32, tag="se")
                den = p4.tile([128, 1], f32, tag="den")
                nc.vector.scalar_tensor_tensor(se[:], fl[:], 1.0, e1[:],
                                               op0=ALU.mult, op1=ALU.mult,
                                               accum_out=den[:])
                rv = p4.tile([128, 1], f32, tag="rv")
                nc.vector.reciprocal(rv[:], den[:])
                csel = p4.tile([128, NE], f32, tag="csel")
                cws = p4.tile([128, 1], f32, tag="cws")
                nc.vector.scalar_tensor_tensor(csel[:], se[:], 1.0, selm[:],
                                               op0=ALU.mult, op1=ALU.mult,
                                               accum_out=cws[:])
                cwn = p4.tile([128, 1], f32r, tag="cwn")
                nc.vector.tensor_tensor(cwn[:], cws[:], rv[:], op=ALU.mult)
                pc = p4tps.tile([1, 128], f32r, tag="cw_ps")
                nc.tensor.transpose(pc[:], cwn[:], ident[:])
                nc.vector.tensor_copy(asf(cwrow[0:1, tsl]), asf(pc[:]))
            nc.gpsimd.partition_broadcast(CW[:], cwrow[:])

        lg_scope.close()

        # ============ Phase 5: expert FFN =================================
        with tc.tile_pool(name="p5w", bufs=2) as p5w, \
             tc.tile_pool(name="p5h", bufs=1) as p5h, \
             tc.tile_pool(name="p5s", bufs=2) as p5s, \
             tc.tile_pool(name="p5ps", bufs=2, space="PSUM") as p5ps:
            for ts in range(NTS):
                sl = slice(ts * TSL, (ts + 1) * TSL)
                hts_ = []
                for it in range(ITI):
                    w1b = p5w.tile([128, HT, 128], f32r, tag="w1b",
                                   bufs=3)
                    nc.sync.dma_start(w1b[:], w1T_d[it])
                    w3b = p5w.tile([128, HT, 128], f32r, tag="w3b",
                                   bufs=3)
                    nc.sync.dma_start(w3b[:], w3T_d[it])
                    pg = p5ps.tile([128, TSL], f32, tag="g")
                    pu = p5ps.tile([128, TSL], f32, tag="u")
                    for ht in range(HT):
                        nc.tensor.matmul(pg[:], w1b[:, ht, :],
                                         x2[ht][:, sl],
                                         start=(ht == 0),
                                         stop=(ht == HT - 1))
                    for ht in range(HT):
                        nc.tensor.matmul(pu[:], w3b[:, ht, :],
                                         x2[ht][:, sl],
                                         start=(ht == 0),
                                         stop=(ht == HT - 1))
                    sg = p5s.tile([128, TSL], f32, tag="sg")
                    nc.scalar.activation(sg[:], pg[:], AF.Silu)
                    ht_ = p5h.tile([128, TSL], f32r, tag=f"h{it}")
                    nc.vector.tensor_tensor(ht_[:], sg[:], pu[:],
                                            op=ALU.mult)
                    hts_.append(ht_)
                for dtt in range(HT):
                    w2b = p5w.tile([128, ITI, 128], f32r, tag="w2b")
                    nc.sync.dma_start(w2b[:], w2T_d[dtt])
                    pd = p5ps.tile([128, TSL], f32, tag="d")
                    for it in range(ITI):
                        nc.tensor.matmul(pd[:], w2b[:, it, :],
                                         hts_[it][:],
                                         start=(it == 0),
                                         stop=(it == ITI - 1))
                    mo = p5s.tile([128, TSL], f32, tag="mo")
                    nc.vector.tensor_tensor(mo[:], pd[:], CW[:, sl],
                                            op=ALU.mult)
                    nc.sync.dma_start(
                        moeT_o[dtt * 128:(dtt + 1) * 128, sl], mo[:])

    nc.compile()
    return nc


def _host_prep(positions, hidden_states, qkv_w, o_w, gate_w, w1, w2, w3,
               ln1_w, ln2_w):
    f = np.float32
    pos = np.asarray(positions).astype(f)
    hidden = np.asarray(hidden_states, dtype=f)
    hT = np.ascontiguousarray(hidden.T)

    half = HD // 2
    inv = THETA ** (-np.arange(half, dtype=f) * 2.0 / HD)
    ang = inv[:, None] * pos[None, :]          # [64, T]
    cos = np.cos(ang).astype(f)
    sin = np.sin(ang).astype(f)
    cos128 = np.concatenate([cos, cos], axis=0)
    # rope(x) = x*cos128 + swap_halves(x)*sinsign, sinsign = [-sin ; +sin]
    sinsign = np.concatenate([-sin, sin], axis=0)
    cosq, sinq = (cos128 * SCALE).astype(f), (sinsign * SCALE).astype(f)
    cosk, sink = cos128.astype(f), sinsign.astype(f)

    qq, kk = np.meshgrid(np.arange(128), np.arange(128), indexing="ij")
    trimask = np.where(kk <= qq, 0.0, -1e30).astype(f)
    ident = np.eye(128, dtype=f)
    ones = np.ones((128, 1), dtype=f)

    qkv_f = (np.asarray(qkv_w, dtype=f) * np.asarray(ln1_w, dtype=f)[None, :])
    gate_f = (np.asarray(gate_w, dtype=f) * np.asarray(ln2_w, dtype=f)[None, :])
    gwT = np.ascontiguousarray(gate_f.T)       # [HID, 8]
    ln2 = np.asarray(ln2_w, dtype=f)

    in_maps = []
    for c in range(N_CORES):
        kvh = c // (NH // NKV)
        qs = qkv_f[c * HD:(c + 1) * HD]                       # [128, HID]
        ks = qkv_f[NH * HD + kvh * HD: NH * HD + (kvh + 1) * HD]
        vs = qkv_f[(NH + NKV) * HD + kvh * HD:
                   (NH + NKV) * HD + (kvh + 1) * HD]
        qkvT = np.ascontiguousarray(
            np.concatenate([qs, ks, vs], axis=0).T)           # [HID, 384]
        owT = np.ascontiguousarray(
            np.asarray(o_w, dtype=f)[c * 128:(c + 1) * 128, :].T)  # [HID,128]
        w1c = np.asarray(w1[c], dtype=f) * ln2[None, :]       # [INTER, HID]
        w3c = np.asarray(w3[c], dtype=f) * ln2[None, :]
        w2c = np.asarray(w2[c], dtype=f)                      # [HID, INTER]
        # [ITI, 128p, HT, 128f]: lhsT tile [p, a, f] loads contiguously
        w1T = np.ascontiguousarray(
            w1c.T.reshape(HT, 128, ITI, 128).transpose(2, 1, 0, 3))
        w3T = np.ascontiguousarray(
            w3c.T.reshape(HT, 128, ITI, 128).transpose(2, 1, 0, 3))
        w2T = np.ascontiguousarray(
            w2c.T.reshape(ITI, 128, HT, 128).transpose(2, 1, 0, 3))
        selmask = np.zeros((128, NE), dtype=f)
        selmask[:, c] = 1.0
        in_maps.append({
            "hT": hT,
            "hslice": np.ascontiguousarray(hT[c * 128:(c + 1) * 128]),
            "qkvT": qkvT, "owT": owT, "gwT": gwT,
            "w1T": w1T, "w3T": w3T, "w2T": w2T,
            "cosq": cosq, "sinq": sinq, "cosk": cosk, "sink": sink,
            "trimask": trimask, "selmask": selmask, "ident": ident,
            "onescol": ones,
        })
    return in_maps


def kernel(positions, hidden_states, qkv_w, o_w, gate_w, w1, w2, w3,
           ln1_w, ln2_w, _trace=False):
    from concourse.bass_utils import run_bass_kernel_spmd
    if "nc" not in _CACHE:
        _CACHE["nc"] = _build_nc()
    nc = _CACHE["nc"]
    in_maps = _host_prep(positions, hidden_states, qkv_w, o_w, gate_w,
                         w1, w2, w3, ln1_w, ln2_w)
    res = run_bass_kernel_spmd(nc, in_maps, list(range(N_CORES)),
                               trace=_trace)
    _CACHE["last_result"] = res
    moeT = np.zeros((HID, T), dtype=np.float64)
    for c in range(N_CORES):
        moeT += res.results[c]["moeT"]
    moe = np.ascontiguousarray(moeT.T).astype(np.float32)
    resid = np.ascontiguousarray(res.results[0]["residT"].T)
    return (moe, resid)


# revision 28
# speedup vs baseline: 1.0762x; 1.0029x over previous
"""Mixtral decoder layer (attention + top-2-of-8 MoE) on 8 trn2 NeuronCores.

Sharding: attention is head-parallel (8 heads -> 1 head/core, GQA kv head =
core//4), o_proj is sharded over output rows; two 1MB AllGathers knit the
cores back together.  The MoE is expert-parallel (8 experts -> 1 expert/core);
each core computes its expert's contribution weighted by the dense top-2
router weight and the host sums the 8 partial outputs.

Device layout is transposed throughout: activations live as [feature, token]
so every matmul contraction dim sits on the SBUF partition axis.  The host
pre-transposes weights (and folds the rmsnorm gains into the adjacent weight
matrices) and un-transposes the outputs.

All matmuls run in float32r (full-rate).  Walrus requires every operand of an
fp32r matmul to be *produced* as fp32r, so matmul-feeding tiles are declared
float32r and written through f32r APs; non-matmul consumers read them through
.bitcast(float32).
"""

import numpy as np

T, HID, NH, NKV, HD = 2048, 1024, 8, 2, 128
INTER, NE, TOPK = 3584, 8, 2
EPS, THETA = 1e-5, 10000.0
N_CORES = 8
HT = HID // 128    # 8 h-tiles
TT = T // 128      # 16 t-tiles
ITI = INTER // 128  # 28 i-tiles
TSL = 512          # token-slice for the FFN phase
NTS = T // TSL
SCALE = HD ** -0.5

_CACHE = {}


def _build_nc():
    import concourse.bacc as bacc
    import concourse.tile as tile
    import concourse.mybir as mybir
    from contextlib import ExitStack

    dt = mybir.dt
    f32 = dt.float32
    f32r = dt.float32r
    AF = mybir.ActivationFunctionType
    ALU = mybir.AluOpType
    AX = mybir.AxisListType

    nc = bacc.Bacc("TRN2", target_bir_lowering=False, debug=False,
                   num_devices=N_CORES)

    # ---- DRAM I/O ----  (tensors feeding matmuls are float32r: same bits)
    hT_d = nc.dram_tensor("hT", [HID, T], f32r, kind="ExternalInput")
    hslice_d = nc.dram_tensor("hslice", [128, T], f32, kind="ExternalInput")
    qkvT_d = nc.dram_tensor("qkvT", [HID, 3 * HD], f32r, kind="ExternalInput")
    owT_d = nc.dram_tensor("owT", [HID, 128], f32r, kind="ExternalInput")
    gwT_d = nc.dram_tensor("gwT", [HID, NE], f32, kind="ExternalInput")
    w1T_d = nc.dram_tensor("w1T", [ITI, 128, HT, 128], f32r,
                           kind="ExternalInput")
    w3T_d = nc.dram_tensor("w3T", [ITI, 128, HT, 128], f32r,
                           kind="ExternalInput")
    w2T_d = nc.dram_tensor("w2T", [HT, 128, ITI, 128], f32r,
                           kind="ExternalInput")
    cosq_d = nc.dram_tensor("cosq", [128, T], f32, kind="ExternalInput")
    sinq_d = nc.dram_tensor("sinq", [128, T], f32, kind="ExternalInput")
    cosk_d = nc.dram_tensor("cosk", [128, T], f32, kind="ExternalInput")
    sink_d = nc.dram_tensor("sink", [128, T], f32, kind="ExternalInput")
    trimask_d = nc.dram_tensor("trimask", [128, 128], f32, kind="ExternalInput")
    selmask_d = nc.dram_tensor("selmask", [128, NE], f32, kind="ExternalInput")
    ident_d = nc.dram_tensor("ident", [128, 128], f32r, kind="ExternalInput")
    ones_d = nc.dram_tensor("onescol", [128, 1], f32r, kind="ExternalInput")
    moeT_o = nc.dram_tensor("moeT", [HID, T], f32, kind="ExternalOutput")
    residT_o = nc.dram_tensor("residT", [HID, T], f32, kind="ExternalOutput")

    def r(ap):
        return ap.bitcast(f32r)

    def asf(ap):
        return ap.bitcast(f32)

    with tile.TileContext(nc) as tc, ExitStack() as top:
        cpool = top.enter_context(tc.tile_pool(name="consts", bufs=1))
        ident = cpool.tile([128, 128], f32r, tag="ident")
        nc.sync.dma_start(ident[:], ident_d[:])
        ones = cpool.tile([128, 1], f32r, tag="ones")
        nc.sync.dma_start(ones[:], ones_d[:])

        # DRAM bounce buffers for collectives (outputs Shared)
        dpool = top.enter_context(tc.tile_pool(name="dram", bufs=1,
                                               space="DRAM"))
        cout1 = nc.dram_tensor("cc_out1", [HID, T], f32r, addr_space="Shared")
        cout2 = nc.dram_tensor("cc_out2", [HID, T], f32, addr_space="Shared")

        attn_scope = ExitStack()
        apool = attn_scope.enter_context(tc.tile_pool(name="attn_act",
                                                      bufs=1))
        qhat = apool.tile([128, T], f32r, tag="qhat")
        khat = apool.tile([128, T], f32r, tag="khat")
        vT = apool.tile([128, T], f32r, tag="vT")

        def rmsnorm_scale(src_tiles, src_f32r, pool, pool1, pspool, tag):
            """src_tiles: 8 [128, T] tiles covering HID on partitions.
            Returns s [1, T] sbuf tile: rsqrt(mean_h(x^2) + eps)."""
            ps = pspool.tile([1, T], f32, tag=f"{tag}_ps")
            for ht in range(HT):
                src = src_tiles[ht][:]
                if src_f32r:
                    src = asf(src)
                for nt in range(T // 512):
                    sl = slice(nt * 512, nt * 512 + 512)
                    sq = pool.tile([128, 512], f32r, tag=f"{tag}_sq")
                    nc.scalar.square(sq[:], src[:, sl])
                    nc.tensor.matmul(ps[0:1, sl], ones[:], sq[:],
                                     start=(ht == 0), stop=(ht == HT - 1))
            epst = pool1.tile([1, 1], f32, tag=f"{tag}_eps")
            nc.gpsimd.memset(epst[:], EPS)
            srt = pool1.tile([1, T], f32, tag=f"{tag}_srt")
            nc.scalar.activation(srt[:], ps[0:1, :], AF.Sqrt,
                                 bias=epst[:], scale=1.0 / HID)
            s = pool1.tile([1, T], f32, tag=f"{tag}_s")
            nc.vector.reciprocal(s[:], srt[:])
            return s

        # ============ Phase 1: x1T = hiddenT * rsqrt(mean h^2+eps) =========
        with tc.tile_pool(name="p1", bufs=2) as p1, \
             tc.tile_pool(name="p1c", bufs=1) as p1c, \
             tc.tile_pool(name="p1x", bufs=1) as p1x:
            hts = []
            for ht in range(HT):
                t_ = p1x.tile([128, T], f32r, tag=f"ht{ht}")
                nc.sync.dma_start(t_[:], hT_d[ht * 128:(ht + 1) * 128, :])
                hts.append(t_)
            with tc.tile_pool(name="p1ps", bufs=1, space="PSUM") as p1ps:
                s1 = rmsnorm_scale(hts, True, p1, p1c, p1ps, "s1")
            p2ps_ctx = tc.tile_pool(name="p2ps", bufs=2, space="PSUM")
            p2ps = p2ps_ctx.__enter__()
            S1 = p1x.tile([128, T], f32, tag="S1")
            nc.gpsimd.partition_broadcast(S1[:], s1[:])
            x1 = hts
            for ht in range(HT):
                # in-place normalize; output written as f32r
                nc.vector.tensor_tensor(x1[ht][:], asf(x1[ht][:]), S1[:],
                                        op=ALU.mult)

            # ============ Phase 2a: qkv + rope ============================
            qkvw = []
            for ht in range(HT):
                t_ = p1c.tile([128, 3 * HD], f32r, tag=f"qkvw{ht}")
                nc.sync.dma_start(t_[:], qkvT_d[ht * 128:(ht + 1) * 128, :])
                qkvw.append(t_)

            def load_rope(cd, sd):
                c_ = p1c.tile([128, T], f32, tag="rope_cos", name="rc")
                nc.sync.dma_start(c_[:], cd[:])
                s_ = p1c.tile([128, T], f32, tag="rope_sin", name="rs")
                nc.sync.dma_start(s_[:], sd[:])
                return c_, s_

            def qkv_mm(col, ps):
                for nt in range(T // 512):
                    sl = slice(nt * 512, nt * 512 + 512)
                    for ht in range(HT):
                        nc.tensor.matmul(
                            ps[:, sl],
                            qkvw[ht][:, col * 128:(col + 1) * 128],
                            x1[ht][:, sl],
                            start=(ht == 0), stop=(ht == HT - 1))

            def rope(ps, cos_t, sinsign_t, dst):
                # rope(x) = x*cos128 + swap_halves(x)*[-sin ; +sin]
                raw = p1c.tile([128, T], f32, tag="rope_raw", name="rr")
                nc.any.tensor_copy(raw[:], ps[:])
                sw = p1c.tile([128, T], f32, tag="rope_sw", name="rw")
                nc.sync.dma_start(sw[0:64, :], raw[64:128, :])
                nc.sync.dma_start(sw[64:128, :], raw[0:64, :])
                for nt in range(4):
                    sl = slice(nt * 512, nt * 512 + 512)
                    t1 = p1.tile([128, 512], f32, tag="rope_t1")
                    t2 = p1.tile([128, 512], f32, tag="rope_t2")
                    nc.vector.tensor_mul(t1[:], raw[:, sl], cos_t[:, sl])
                    nc.vector.tensor_mul(t2[:], sw[:, sl], sinsign_t[:, sl])
                    nc.vector.tensor_tensor(dst[:, sl], t1[:], t2[:],
                                            op=ALU.add)

            psq = p2ps.tile([128, T], f32, tag="qkv")
            qkv_mm(0, psq)
            cq_t, sq_t = load_rope(cosq_d, sinq_d)
            rope(psq, cq_t, sq_t, qhat)
            psk = p2ps.tile([128, T], f32, tag="qkv")
            qkv_mm(1, psk)
            ck_t, sk_t = load_rope(cosk_d, sink_d)
            rope(psk, ck_t, sk_t, khat)
            psv = p2ps.tile([128, T], f32, tag="qkv")
            qkv_mm(2, psv)
            nc.any.tensor_copy(vT[:], psv[:])
            p2ps_ctx.__exit__(None, None, None)

        # ============ Phase 2b: scores/softmax/PV =========================
        aop = attn_scope.enter_context(tc.tile_pool(name="aop", bufs=1))
        aoT = aop.tile([128, T], f32r, tag="aoT")
        with tc.tile_pool(name="p2b", bufs=2) as p2b, \
             tc.tile_pool(name="p2bx", bufs=1) as p2bx, \
             tc.tile_pool(name="sps", bufs=1, space="PSUM") as sps, \
             tc.tile_pool(name="tps", bufs=2, space="PSUM") as tps:
            trimask = p2bx.tile([128, 128], f32, tag="trimask")
            nc.sync.dma_start(trimask[:], trimask_d[:])
            vN = []
            for kt in range(TT):
                ps = tps.tile([128, 128], f32r, tag="pt_ps")
                nc.tensor.transpose(ps[:], vT[:, kt * 128:(kt + 1) * 128],
                                    ident[:])
                t_ = p2bx.tile([128, 128], f32r, tag=f"vN{kt}")
                nc.vector.tensor_copy(t_[:], ps[:])
                vN.append(t_)

            for qg in range(TT // 4):
                nkt = 4 * qg + 4
                PTs = [p2bx.tile([128, 512], f32r, tag=f"PT{kt}",
                                 name=f"PT{kt}_{qg}", bufs=2)
                       for kt in range(nkt)]
                rrow = p2b.tile([1, 512], f32, tag="rrow")
                for qi in range(4 * qg, 4 * qg + 4):
                    nk = (qi + 1) * 128
                    pss = sps.tile([128, T], f32, tag="scores")
                    for skx in range(0, nk, 512):
                        wk = min(512, nk - skx)
                        nc.tensor.matmul(
                            pss[:, skx:skx + wk],
                            qhat[:, qi * 128:(qi + 1) * 128],
                            khat[:, skx:skx + wk],
                            start=True, stop=True)
                    dsl = slice(qi * 128, (qi + 1) * 128)
                    nc.vector.tensor_add(pss[:, dsl], pss[:, dsl],
                                         trimask[:])
                    nm = p2b.tile([128, 1], f32, tag="nm")
                    nc.vector.reduce_max(nm[:], pss[:, 0:nk], axis=AX.X,
                                         negate=True)
                    probs = p2b.tile([128, T], f32r, tag="probs")
                    rsum = p2b.tile([128, 1], f32, tag="rsum")
                    nc.scalar.activation(probs[:, 0:nk], pss[:, 0:nk],
                                         AF.Exp, bias=nm[:], scale=1.0,
                                         accum_out=rsum[:])
                    rinv = p2b.tile([128, 1], f32, tag="rinv")
                    nc.vector.reciprocal(rinv[:], rsum[:])
                    rinv_r = p2b.tile([128, 1], f32r, tag="rinv_r")
                    nc.vector.tensor_copy(rinv_r[:], rinv[:])
                    pr = tps.tile([1, 128], f32r, tag="pt_ps")
                    nc.tensor.transpose(pr[:], rinv_r[:], ident[:])
                    nc.vector.tensor_copy(
                        asf(rrow[0:1, (qi % 4) * 128:(qi % 4 + 1) * 128]),
                        asf(pr[:]))
                    for kt in range(qi + 1):
                        pt = tps.tile([128, 128], f32r, tag="pt_ps")
                        nc.tensor.transpose(
                            pt[:], probs[:, kt * 128:(kt + 1) * 128],
                            ident[:])
                        nc.vector.tensor_copy(
                            PTs[kt][:, (qi % 4) * 128:(qi % 4 + 1) * 128],
                            pt[:])
                RB = p2b.tile([128, 512], f32, tag="RB")
                nc.gpsimd.partition_broadcast(RB[:], rrow[:])
                pso = tps.tile([128, 512], f32, tag="av")
                for kt in range(nkt):
                    c0 = max(0, (kt - 4 * qg) * 128)
                    nc.tensor.matmul(pso[:, c0:512], vN[kt][:],
                                     PTs[kt][:, c0:512],
                                     start=(kt == 0), stop=(kt == nkt - 1),
                                     skip_group_check=True)
                nc.vector.tensor_tensor(aoT[:, qg * 512:(qg + 1) * 512],
                                        pso[:], RB[:], op=ALU.mult)

        # ============ Phase 2c: AllGather heads + o_proj + residual =======
        with tc.tile_pool(name="p2c", bufs=1) as p2c, \
             tc.tile_pool(name="ops", bufs=1, space="PSUM") as ops:
            cin1 = dpool.tile([128, T], f32r, tag="cin1")
            nc.sync.dma_start(cin1[:], aoT[:])
            nc.gpsimd.collective_compute(
                "AllGather", mybir.AluOpType.bypass,
                replica_groups=[list(range(N_CORES))],
                ins=[cin1.opt()], outs=[cout1.ap()])
            aof = []
            for rt in range(HT):
                t_ = p2c.tile([128, T], f32r, tag=f"aof{rt}")
                nc.sync.dma_start(t_[:], cout1[rt * 128:(rt + 1) * 128, :])
                aof.append(t_)
            oww = []
            for rt in range(HT):
                t_ = p2c.tile([128, 128], f32r, tag=f"oww{rt}")
                nc.sync.dma_start(t_[:], owT_d[rt * 128:(rt + 1) * 128, :])
                oww.append(t_)
            pso = ops.tile([128, T], f32, tag="o")
            for nt in range(T // 512):
                sl = slice(nt * 512, nt * 512 + 512)
                for rt in range(HT):
                    nc.tensor.matmul(pso[:, sl], oww[rt][:],
                                     aof[rt][:, sl],
                                     start=(rt == 0), stop=(rt == HT - 1))
            hsl = p2c.tile([128, T], f32, tag="hsl")
            nc.sync.dma_start(hsl[:], hslice_d[:])
            rsl = p2c.tile([128, T], f32, tag="rsl")
            nc.vector.tensor_add(rsl[:], pso[:], hsl[:])
            cin2 = dpool.tile([128, T], f32, tag="cin2")
            nc.sync.dma_start(cin2[:], rsl[:])
            nc.gpsimd.collective_compute(
                "AllGather", mybir.AluOpType.bypass,
                replica_groups=[list(range(N_CORES))],
                ins=[cin2.opt()], outs=[cout2.ap()])
            nc.sync.dma_start(residT_o[:], cout2.ap())

        attn_scope.close()

        # ============ Phase 3: x2T + fp32 router logits ===================
        x2pool = top.enter_context(tc.tile_pool(name="x2", bufs=1))
        cwpool = top.enter_context(tc.tile_pool(name="cw", bufs=1))
        CW = cwpool.tile([128, T], f32, tag="CW")
        lg_scope = ExitStack()
        lgpool = lg_scope.enter_context(tc.tile_pool(name="lgp", bufs=1))
        lgT = lgpool.tile([NE, T], f32, tag="lgT")
        x2 = []
        with tc.tile_pool(name="p3", bufs=2) as p3, \
             tc.tile_pool(name="p3x", bufs=1) as p3x, \
             tc.tile_pool(name="p3ps", bufs=1, space="PSUM") as p3ps, \
             tc.tile_pool(name="p3lps", bufs=1, space="PSUM") as p3lps:
            rts = []
            for ht in range(HT):
                t_ = p3x.tile([128, T], f32, tag=f"rt{ht}")
                nc.sync.dma_start(t_[:], cout2[ht * 128:(ht + 1) * 128, :])
                rts.append(t_)
            s2 = rmsnorm_scale(rts, False, p3, p3x, p3ps, "s2")
            S2 = p3x.tile([128, T], f32, tag="S2")
            nc.gpsimd.partition_broadcast(S2[:], s2[:])
            # router logits in full fp32 on the *unnormalized* residual
            # (the rmsnorm scale s2[t] commutes with the h-contraction)
            gww = []
            for ht in range(HT):
                t_ = p3x.tile([128, NE], f32, tag=f"gww{ht}")
                nc.sync.dma_start(t_[:], gwT_d[ht * 128:(ht + 1) * 128, :])
                gww.append(t_)
            psl = p3lps.tile([NE, T], f32, tag="lg")
            for nt in range(T // 512):
                sl = slice(nt * 512, nt * 512 + 512)
                for ht in range(HT):
                    nc.tensor.matmul(psl[:, sl], gww[ht][:],
                                     rts[ht][:, sl],
                                     start=(ht == 0), stop=(ht == HT - 1))
            s2b8 = p3x.tile([NE, T], f32, tag="s2b8")
            nc.gpsimd.partition_broadcast(s2b8[:], s2[:])
            nc.vector.tensor_tensor(lgT[:], psl[:], s2b8[:], op=ALU.mult)
            for ht in range(HT):
                t_ = x2pool.tile([128, T], f32r, tag=f"x2{ht}")
                nc.vector.tensor_tensor(t_[:], rts[ht][:], S2[:],
                                        op=ALU.mult)
                x2.append(t_)

        # ============ Phase 4: router -> CW [128, T] ======================
        with tc.tile_pool(name="p4", bufs=2) as p4, \
             tc.tile_pool(name="p4x", bufs=1) as p4x, \
             tc.tile_pool(name="p4tps", bufs=2, space="PSUM") as p4tps:
            selm = p4x.tile([128, NE], f32, tag="selm")
            nc.sync.dma_start(selm[:], selmask_d[:])
            cwrow = p4x.tile([1, T], f32, tag="cwrow")
            for tt in range(TT):
                tsl = slice(tt * 128, (tt + 1) * 128)
                p8 = p4tps.tile([128, NE], f32, tag="lgt_ps")
                nc.tensor.transpose(p8[:], lgT[:, tsl],
                                    asf(ident[0:NE, 0:NE]))
                lgN = p4.tile([128, NE], f32, tag="lgN")
                nc.vector.tensor_copy(lgN[:], p8[:])
                m1 = p4.tile([128, 1], f32, tag="m1")
                nc.vector.reduce_max(m1[:], lgN[:], axis=AX.X)
                eq = p4.tile([128, NE], f32, tag="eq")
                nc.vector.tensor_scalar(eq[:], lgN[:], m1[:], None,
                                        op0=ALU.is_equal)
                msk = p4.tile([128, NE], f32, tag="msk")
                nc.vector.scalar_tensor_tensor(
                    msk[:], eq[:], -1e30, lgN[:],
                    op0=ALU.mult, op1=ALU.add)
                m2 = p4.tile([128, 1], f32, tag="m2")
                nc.vector.reduce_max(m2[:], msk[:], axis=AX.X)
                fl = p4.tile([128, NE], f32, tag="fl")
                nc.vector.tensor_scalar(fl[:], lgN[:], m2[:], None,
                                        op0=ALU.is_ge)
                dd = p4.tile([128, NE], f32, tag="dd")
                nc.vector.tensor_scalar(dd[:], lgN[:], m1[:], None,
                                        op0=ALU.subtract)
                e1 = p4.tile([128, NE], f32, tag="e1")
                nc.scalar.activation(e1[:], dd[:], AF.Exp)
                se = p4.tile([128, NE], f32, tag="se")
                den = p4.tile([128, 1], f32, tag="den")
                nc.vector.scalar_tensor_tensor(se[:], fl[:], 1.0, e1[:],
                                               op0=ALU.mult, op1=ALU.mult,
                                               accum_out=den[:])
                rv = p4.tile([128, 1], f32, tag="rv")
                nc.vector.reciprocal(rv[:], den[:])
                csel = p4.tile([128, NE], f32, tag="csel")
                cws = p4.tile([128, 1], f32, tag="cws")
                nc.vector.scalar_tensor_tensor(csel[:], se[:], 1.0, selm[:],
                                               op0=ALU.mult, op1=ALU.mult,
                                               accum_out=cws[:])
                cwn = p4.tile([128, 1], f32r, tag="cwn")
                nc.vector.tensor_tensor(cwn[:], cws[:], rv[:], op=ALU.mult)
                pc = p4tps.tile([1, 128], f32r, tag="cw_ps")
                nc.tensor.transpose(pc[:], cwn[:], ident[:])
                nc.vector.tensor_copy(asf(cwrow[0:1, tsl]), asf(pc[:]))
            nc.gpsimd.partition_broadcast(CW[:], cwrow[:])

        lg_scope.close()

        # ============ Phase 5: expert FFN =================================
        with tc.tile_pool(name="p5w", bufs=2) as p5w, \
             tc.tile_pool(name="p5h", bufs=1) as p5h, \
             tc.tile_pool(name="p5s", bufs=2) as p5s, \
             tc.tile_pool(name="p5ps", bufs=2, space="PSUM") as p5ps:
            for ts in range(NTS):
                sl = slice(ts * TSL, (ts + 1) * TSL)
                hts_ = []
                for it in range(ITI):
                    w1b = p5w.tile([128, HT, 128], f32r, tag="w1b",
                                   bufs=3)
                    nc.sync.dma_start(w1b[:], w1T_d[it])
                    w3b = p5w.tile([128, HT, 128], f32r, tag="w3b",
                                   bufs=3)
                    nc.sync.dma_start(w3b[:], w3T_d[it])
                    pg = p5ps.tile([128, TSL], f32, tag="g")
                    pu = p5ps.tile([128, TSL], f32, tag="u")
                    for ht in range(HT):
                        nc.tensor.matmul(pg[:], w1b[:, ht, :],
                                         x2[ht][:, sl],
                                         start=(ht == 0),
                                         stop=(ht == HT - 1))
                    for ht in range(HT):
                        nc.tensor.matmul(pu[:], w3b[:, ht, :],
                                         x2[ht][:, sl],
                                         start=(ht == 0),
                                         stop=(ht == HT - 1))
                    sg = p5s.tile([128, TSL], f32, tag="sg")
                    nc.scalar.activation(sg[:], pg[:], AF.Silu)
                    ht_ = p5h.tile([128, TSL], f32r, tag=f"h{it}")
                    nc.vector.tensor_tensor(ht_[:], sg[:], pu[:],
                                            op=ALU.mult)
                    hts_.append(ht_)
                for dtt in range(HT):
                    w2b = p5w.tile([128, ITI, 128], f32r, tag="w2b")
                    nc.sync.dma_start(w2b[:], w2T_d[dtt])
                    pd = p5ps.tile([128, TSL], f32, tag="d")
                    for it in range(ITI):
                        nc.tensor.matmul(pd[:], w2b[:, it, :],
                                         hts_[it][:],
                                         start=(it == 0),
                                         stop=(it == ITI - 1))
                    mo = p5s.tile([128, TSL], f32, tag="mo")
                    nc.vector.tensor_tensor(mo[:], pd[:], CW[:, sl],
                                            op=ALU.mult)
                    nc.sync.dma_start(
                        moeT_o[dtt * 128:(dtt + 1) * 128, sl], mo[:])

    nc.compile()
    return nc


def _host_prep(positions, hidden_states, qkv_w, o_w, gate_w, w1, w2, w3,
               ln1_w, ln2_w):
    f = np.float32
    pos = np.asarray(positions).astype(f)
    hidden = np.asarray(hidden_states, dtype=f)
    hT = np.ascontiguousarray(hidden.T)

    half = HD // 2
    inv = THETA ** (-np.arange(half, dtype=f) * 2.0 / HD)
    ang = inv[:, None] * pos[None, :]          # [64, T]
    cos = np.cos(ang).astype(f)
    sin = np.sin(ang).astype(f)
    cos128 = np.concatenate([cos, cos], axis=0)
    # rope(x) = x*cos128 + swap_halves(x)*sinsign, sinsign = [-sin ; +sin]
    sinsign = np.concatenate([-sin, sin], axis=0)
    cosq, sinq = (cos128 * SCALE).astype(f), (sinsign * SCALE).astype(f)
    cosk, sink = cos128.astype(f), sinsign.astype(f)

    qq, kk = np.meshgrid(np.arange(128), np.arange(128), indexing="ij")
    trimask = np.where(kk <= qq, 0.0, -1e30).astype(f)
    ident = np.eye(128, dtype=f)
    ones = np.ones((128, 1), dtype=f)

    qkv_f = (np.asarray(qkv_w, dtype=f) * np.asarray(ln1_w, dtype=f)[None, :])
    gate_f = (np.asarray(gate_w, dtype=f) * np.asarray(ln2_w, dtype=f)[None, :])
    gwT = np.ascontiguousarray(gate_f.T)       # [HID, 8]
    ln2 = np.asarray(ln2_w, dtype=f)

    in_maps = []
    for c in range(N_CORES):
        kvh = c // (NH // NKV)
        qs = qkv_f[c * HD:(c + 1) * HD]                       # [128, HID]
        ks = qkv_f[NH * HD + kvh * HD: NH * HD + (kvh + 1) * HD]
        vs = qkv_f[(NH + NKV) * HD + kvh * HD:
                   (NH + NKV) * HD + (kvh + 1) * HD]
        qkvT = np.ascontiguousarray(
            np.concatenate([qs, ks, vs], axis=0).T)           # [HID, 384]
        owT = np.ascontiguousarray(
            np.asarray(o_w, dtype=f)[c * 128:(c + 1) * 128, :].T)  # [HID,128]
        w1c = np.asarray(w1[c], dtype=f) * ln2[None, :]       # [INTER, HID]
        w3c = np.asarray(w3[c], dtype=f) * ln2[None, :]
        w2c = np.asarray(w2[c], dtype=f)                      # [HID, INTER]
        # [ITI, 128p, HT, 128f]: lhsT tile [p, a, f] loads contiguously
        w1T = np.ascontiguousarray(
            w1c.T.reshape(HT, 128, ITI, 128).transpose(2, 1, 0, 3))
        w3T = np.ascontiguousarray(
            w3c.T.reshape(HT, 128, ITI, 128).transpose(2, 1, 0, 3))
        w2T = np.ascontiguousarray(
            w2c.T.reshape(ITI, 128, HT, 128).transpose(2, 1, 0, 3))
        selmask = np.zeros((128, NE), dtype=f)
        selmask[:, c] = 1.0
        in_maps.append({
            "hT": hT,
            "hslice": np.ascontiguousarray(hT[c * 128:(c + 1) * 128]),
            "qkvT": qkvT, "owT": owT, "gwT": gwT,
            "w1T": w1T, "w3T": w3T, "w2T": w2T,
            "cosq": cosq, "sinq": sinq, "cosk": cosk, "sink": sink,
            "trimask": trimask, "selmask": selmask, "ident": ident,
            "onescol": ones,
        })
    return in_maps


def kernel(positions, hidden_states, qkv_w, o_w, gate_w, w1, w2, w3,
           ln1_w, ln2_w, _trace=False):
    from concourse.bass_utils import run_bass_kernel_spmd
    if "nc" not in _CACHE:
        _CACHE["nc"] = _build_nc()
    nc = _CACHE["nc"]
    in_maps = _host_prep(positions, hidden_states, qkv_w, o_w, gate_w,
                         w1, w2, w3, ln1_w, ln2_w)
    res = run_bass_kernel_spmd(nc, in_maps, list(range(N_CORES)),
                               trace=_trace)
    _CACHE["last_result"] = res
    moeT = np.zeros((HID, T), dtype=np.float64)
    for c in range(N_CORES):
        moeT += res.results[c]["moeT"]
    moe = np.ascontiguousarray(moeT.T).astype(np.float32)
    resid = np.ascontiguousarray(res.results[0]["residT"].T)
    return (moe, resid)
